# revision 1
# baseline (speedup 1.0000x reference)
"""GATv2 3-layer GNN on 8 Trainium2 NeuronCores (Bass/Tile) — v2.

Key structure (per core):
  - Nodes are host-binned into 8*49=392 blocks of 128 slots with balanced
    in-degree per block (TB tiles of 128 edge slots per block, no +1 pad).
  - Layer 0: every core builds the FULL xl0 table locally from a
    pre-transposed bf16 copy of x (no AllGather for layer 0).
  - Layers 1,2: xl shards are exchanged with a 2-chunk AllGather
    (blocks 0..CHA-1 early, rest late) so most of the exchange hides
    behind phase-B work of the producing layer.
  - Phase B is software-pipelined: stage1 (mask build, xr-expand
    matmuls, batched indirect gather-accumulate of xl[src]) runs LAG
    blocks ahead of stage2 (edge math, one-hot aggregation, node
    update + fused projection of the NEXT layer).
  - The per-block gather is ONE indirect DMA with a [128, TB] offset
    AP (CCE add onto the xr-expansion), not TB separate calls.
  - bias trick: the table holds x@Wl WITHOUT bias; bl is folded into
    xr's bias (v = xl'+xr' is unchanged) and into the output bias
    (out = num/den - xr + (bias_out + bl)).
"""

import sys

if "/opt/trn_rl_repo" not in sys.path:
    sys.path.insert(0, "/opt/trn_rl_repo")

import numpy as np
import ml_dtypes

BF16 = ml_dtypes.bfloat16

NEG_SLOPE = 0.2
N_NODES = 50000
N_EDGES = 800000
N_GRAPHS = 64
IN_CH = 128
HIDDEN = 128
HEADS = 4
OUT_CH = 64
NCORES = 8


def make_cfg(n_nodes=N_NODES, n_graphs=N_GRAPHS, tb=None, in_ch=IN_CH):
    npc = n_nodes // NCORES
    assert npc * NCORES == n_nodes
    nblk = (npc + 127) // 128
    np_pad = nblk * 128
    cha = max(1, (nblk * 3) // 5)  # early AG chunk (blocks [0, cha))
    return dict(
        N=n_nodes,
        G=n_graphs,
        NPC=npc,
        NP=np_pad,
        NBLK=nblk,
        CHA=cha,
        CHB=nblk - cha,
        GBLK=NCORES * nblk,
        TROWS=NCORES * np_pad,
        TB=tb,
        IN_CH=in_ch,
        LAYERS=[
            (in_ch, HIDDEN, HEADS, HIDDEN // HEADS, True),
            (HIDDEN, HIDDEN, HEADS, HIDDEN // HEADS, True),
            (HIDDEN, OUT_CH, 1, OUT_CH, False),
        ],
    )


# ---------------------------------------------------------------- host prep
def _balanced_bins(deg, nbins, binsz):
    """Assign nodes to bins (each bin holds exactly binsz nodes) minimizing
    max total degree per bin.  Greedy: degree-desc, min-load non-full bin.
    Returns slot_of[node] = bin*binsz + position."""
    import heapq

    n = deg.shape[0]
    order = np.argsort(-deg, kind="stable")
    heap = [(0, b) for b in range(nbins)]
    heapq.heapify(heap)
    fill = np.zeros(nbins, np.int64)
    load = np.zeros(nbins, np.int64)
    slot_of = np.empty(n, np.int64)
    for nd in order:
        while True:
            l, b = heapq.heappop(heap)
            if fill[b] < binsz:
                break
        slot_of[nd] = b * binsz + fill[b]
        fill[b] += 1
        load[b] += deg[nd]
        if fill[b] < binsz:
            heapq.heappush(heap, (load[b], b))
    return slot_of, int(load.max())


def tabrow_of_slot(cfg, slot):
    """Map global slot id -> table row (2-chunk AllGather layout)."""
    NP, NBLK, CHA = cfg["NP"], cfg["NBLK"], cfg["CHA"]
    c = slot // NP
    loc = slot % NP
    b = loc // 128
    r = loc % 128
    rowsA = NCORES * CHA * 128
    return np.where(
        b < CHA,
        c * CHA * 128 + b * 128 + r,
        rowsA + c * (NBLK - CHA) * 128 + (b - CHA) * 128 + r,
    )


def prep(cfg, x, edge_index, batch):
    NPC, NP, NBLK, G, CHA = cfg["NPC"], cfg["NP"], cfg["NBLK"], cfg["G"], cfg["CHA"]
    GBLK = cfg["GBLK"]
    Din = cfg["IN_CH"]
    src = np.asarray(edge_index[0], dtype=np.int64)
    dst = np.asarray(edge_index[1], dtype=np.int64)
    batch = np.asarray(batch, dtype=np.int64)
    x = np.asarray(x, dtype=np.float32)
    N = x.shape[0]

    deg = np.bincount(dst, minlength=N)
    slot_of, maxload = _balanced_bins(deg, GBLK, 128)
    tb = (maxload + 127) // 128
    if cfg["TB"] is None:
        cfg["TB"] = tb
    else:
        assert cfg["TB"] >= tb
    TB = cfg["TB"]
    EB = TB * 128

    node_of_slot = np.full(GBLK * 128, -1, np.int64)
    node_of_slot[slot_of] = np.arange(N)

    # permuted x, laid out in TABLE-ROW block order, transposed per block:
    # xfullT[tb*128 + d, j] = x[node at table row tb*128+j, d]
    x_slot = np.zeros((GBLK * 128, Din), np.float32)
    valid = node_of_slot >= 0
    x_slot[valid] = x[node_of_slot[valid]]
    tabrow = np.asarray(tabrow_of_slot(cfg, np.arange(GBLK * 128)))
    x_tab = np.zeros_like(x_slot)
    x_tab[tabrow] = x_slot  # row = table row
    assert GBLK % 8 == 0
    xfullT = (
        x_tab.reshape(GBLK // 8, 8, 128, Din)
        .transpose(0, 3, 1, 2)
        .reshape((GBLK // 8) * Din, 8 * 128)
    ).astype(BF16)

    # edges
    sd = slot_of[dst]
    ss = slot_of[src]
    trow = np.asarray(tabrow_of_slot(cfg, ss))
    core_of = sd // NP
    dloc = sd % NP
    bloc = dloc // 128
    drow = dloc % 128

    maps = []
    for c in range(NCORES):
        sel = core_of == c
        es_trow = trow[sel]
        ed = drow[sel]
        eb = bloc[sel]
        order = np.argsort(eb * 129 + ed, kind="stable")
        es_trow, ed, eb = es_trow[order], ed[order], eb[order]

        src_idx = np.zeros((NBLK, EB), np.int32)
        dst_row = np.full((NBLK, EB), -1.0, np.float32)
        cnts = np.bincount(eb, minlength=NBLK)
        offs = np.concatenate([[0], np.cumsum(cnts)])
        assert cnts.max() <= EB
        for b in range(NBLK):
            k = cnts[b]
            sl = slice(offs[b], offs[b] + k)
            src_idx[b, :k] = es_trow[sl]
            dst_row[b, :k] = ed[sl].astype(np.float32)

        dcol = dst_row.reshape(NBLK, TB, 128).transpose(0, 2, 1)
        sidx = src_idx.reshape(NBLK, TB, 128).transpose(0, 2, 1)

        # own x^T blocks (for the xr projection pass), in own-block order
        own_tabrows = np.asarray(
            tabrow_of_slot(cfg, c * NP + np.arange(NP))
        )  # slot -> table row for this core's slots
        xownT = (
            x_tab[own_tabrows]
            .reshape(NBLK, 128, Din)
            .transpose(0, 2, 1)
            .reshape(NBLK * Din, 128)
        ).astype(BF16)

        # pool mask [NP, G] over own slots
        pm = np.zeros((NP, G), np.float32)
        own_nodes = node_of_slot[c * NP : (c + 1) * NP]
        vv = own_nodes >= 0
        pm[np.arange(NP)[vv], batch[own_nodes[vv]]] = 1.0

        maps.append(
            dict(
                xfullT=xfullT,
                xownT=xownT,
                src_idx=sidx.reshape(NBLK * 128, TB).astype(np.int32),
                dst_col=dcol.reshape(NBLK * 128, TB).astype(np.float32),
                dst_row=dst_row.astype(BF16),
                pool_mask=pm.astype(BF16),
            )
        )

    counts = np.bincount(batch, minlength=G).astype(np.float32)
    return maps, counts


def prep_weights(cfg, inp):
    w = {}
    for l in range(3):
        Wl = np.asarray(inp[f"Wl{l}"], np.float32)
        bl = np.asarray(inp[f"bl{l}"], np.float32)
        Wr = np.asarray(inp[f"Wr{l}"], np.float32)
        br = np.asarray(inp[f"br{l}"], np.float32)
        att = np.asarray(inp[f"att{l}"], np.float32)
        bo = np.asarray(inp[f"bias{l}"], np.float32)
        D = Wl.shape[1]
        # table holds x@Wl (no bias); xr bias = bl+br; out bias += bl
        w[f"wcat{l}"] = np.concatenate([Wl, Wr], axis=1).astype(BF16)  # [Din,2D]
        w[f"bias_r{l}"] = np.broadcast_to((bl + br)[None, :], (128, D)).copy()
        w[f"bias_out{l}"] = np.broadcast_to((bo + bl)[None, :], (128, D)).copy()
    w["iota_col"] = np.arange(128, dtype=np.float32)[:, None]
    w["iota_sm"] = np.broadcast_to(
        np.arange(128, dtype=np.float32)[None, :], (128, 128)
    ).astype(BF16)
    TB = cfg["TB"]
    for l in range(3):
        D = [HIDDEN, HIDDEN, OUT_CH][l]
        w[f"att{l}r"] = np.broadcast_to(
            np.asarray(inp[f"att{l}"], np.float32).reshape(1, 1, D), (128, TB, D)
        ).reshape(128, TB * D).astype(BF16)
    w["ident"] = np.eye(128, dtype=np.float32)
    return w


# ---------------------------------------------------------------- device build
def build(cfg):
    from concourse import bass, bacc, mybir
    import concourse.tile as tile
    from concourse.tile import add_dep_helper

    F32 = mybir.dt.float32
    BF = mybir.dt.bfloat16
    F8 = mybir.dt.float8e4
    I32 = mybir.dt.int32
    A = mybir.AluOpType
    ACTF = mybir.ActivationFunctionType

    NP, NBLK, TB, TROWS, G = cfg["NP"], cfg["NBLK"], cfg["TB"], cfg["TROWS"], cfg["G"]
    CHA, CHB, GBLK = cfg["CHA"], cfg["CHB"], cfg["GBLK"]
    Din0 = cfg["IN_CH"]
    EB = TB * 128
    LAYERS = cfg["LAYERS"]
    LAG = 2

    nc = bacc.Bacc(
        "TRN2",
        target_bir_lowering=False,
        debug=False,
        enable_asserts=False,
        num_devices=NCORES,
    )

    ext = {}

    def ein(name, shape, dt):
        ext[name] = nc.dram_tensor(name, shape, dt, kind="ExternalInput").ap()
        return ext[name]

    xfullT = ein("xfullT", [(GBLK // 8) * Din0, 8 * 128], BF)
    xownT = ein("xownT", [NBLK * Din0, 128], BF)
    src_idx = ein("src_idx", [NBLK * 128, TB], I32)
    dst_col = ein("dst_col", [NBLK * 128, TB], F32)
    dst_row = ein("dst_row", [NBLK, EB], BF)
    pool_mask = ein("pool_mask", [NP, G], BF)
    iota_col_d = ein("iota_col", [128, 1], F32)
    iota_sm_d = ein("iota_sm", [128, 128], BF)
    ident_d = ein("ident", [128, 128], F32)
    wcat_d, biasr_d, att_d, biasout_d = [], [], [], []
    for l, (Din, D, H, C, _) in enumerate(LAYERS):
        wcat_d.append(ein(f"wcat{l}", [Din, 2 * D], BF))
        biasr_d.append(ein(f"bias_r{l}", [128, D], F32))
        att_d.append(ein(f"att{l}r", [128, TB * D], BF))
        biasout_d.append(ein(f"bias_out{l}", [128, D], F32))

    pool_out = nc.dram_tensor("pool_out", [G, OUT_CH], F32, kind="ExternalOutput").ap()

    # internal DRAM
    tabs = []
    ccA, ccB = [None] * 3, [None] * 3
    for l, (Din, D, H, C, _) in enumerate(LAYERS):
        tabs.append(
            nc.dram_tensor(
                f"tab{l}", [TROWS, D], F8, kind="Internal", addr_space="Shared"
            ).ap()
        )
        if l >= 1:
            ccA[l] = nc.dram_tensor(f"ccA{l}", [CHA * 128, D], F8, kind="Internal").ap()
            ccB[l] = nc.dram_tensor(f"ccB{l}", [CHB * 128, D], F8, kind="Internal").ap()

    from contextlib import ExitStack

    with tile.TileContext(nc) as tc, ExitStack() as pools:
        const = pools.enter_context(tc.tile_pool(name="const", bufs=1))
        s1 = pools.enter_context(tc.tile_pool(name="s1", bufs=LAG + 2))
        s1b = pools.enter_context(tc.tile_pool(name="s1b", bufs=2))
        s2 = pools.enter_context(tc.tile_pool(name="s2", bufs=2))
        nodep = pools.enter_context(tc.tile_pool(name="nodep", bufs=2))
        psum_exp = pools.enter_context(tc.tile_pool(name="psum_exp", bufs=2, space="PSUM"))
        psum_agg = pools.enter_context(tc.tile_pool(name="psum_agg", bufs=2, space="PSUM"))
        psum_pam = pools.enter_context(tc.tile_pool(name="psum_pam", bufs=2, space="PSUM"))
        psum_pat = pools.enter_context(tc.tile_pool(name="psum_pat", bufs=1, space="PSUM"))
        psum_pool = pools.enter_context(tc.tile_pool(name="psum_pool", bufs=1, space="PSUM"))

        # persistent SBUF: xr tables (double-buffered across layers)
        xr_sb = [
            nc.alloc_sbuf_tensor(f"xr_sb{k}", [128, NBLK, HIDDEN], BF).ap()
            for k in range(2)
        ]

        def const_tile(shape, dt, src_ap, tag):
            t = const.tile(shape, dt, tag=tag)
            nc.sync.dma_start(out=t[:], in_=src_ap)
            return t

        ident = const_tile([128, 128], F32, ident_d[:], "ident")
        iota_c = const_tile([128, 1], F32, iota_col_d[:], "iotac")
        iota_sm = const_tile([128, 128], BF, iota_sm_d[:], "iotasm")
        dummy = const_tile([1, 1], F32, ident_d[:1, :1], "dummy")
        wcat_s, biasr_s, att_s, biasout_s = [], [], [], []
        for l, (Din, D, H, C, _) in enumerate(LAYERS):
            wcat_s.append(const_tile([Din, 2 * D], BF, wcat_d[l][:], f"wc{l}"))
            biasr_s.append(const_tile([128, D], F32, biasr_d[l][:], f"br{l}"))
            att_s.append(const_tile([128, TB * D], BF, att_d[l][:], f"at{l}"))
            biasout_s.append(const_tile([128, D], F32, biasout_d[l][:], f"bo{l}"))

        # ============ layer 0: local full-table build + own xr pass
        # batched 8 blocks per DMA: one strided load, 8 matmuls, one store.
        D0 = LAYERS[0][1]
        tab0_writes = []
        GRP = 8
        assert GBLK % GRP == 0
        for gg in range(GBLK // GRP):
            xT8 = nodep.tile([Din0, GRP, 128], BF, tag="t0_xT")
            nc.sync.dma_start(
                out=xT8[:],
                in_=xfullT[gg * Din0 : (gg + 1) * Din0, :],
            )
            xl8 = nodep.tile([128, GRP, D0], F8, tag="t0_xl")
            for k in range(GRP):
                pp_f = psum_pam.tile([128, 2 * HIDDEN], F32, tag="pa_mm")
                pp = pp_f[:, :D0]
                nc.tensor.matmul(
                    out=pp, lhsT=xT8[:, k, :], rhs=wcat_s[0][:, :D0],
                    start=True, stop=True
                )
                nc.scalar.copy(out=xl8[:, k, :], in_=pp)
            wi = nc.sync.dma_start(
                out=tabs[0][gg * GRP * 128 : (gg + 1) * GRP * 128, :].rearrange(
                    "(g p) d -> p g d", g=GRP
                ),
                in_=xl8[:],
            )
            tab0_writes.append(wi)

        for b in range(NBLK):
            xT = nodep.tile([Din0, 128], BF, tag="own_xT")
            nc.sync.dma_start(out=xT[:], in_=xownT[b * Din0 : (b + 1) * Din0, :])
            pr_f = psum_pam.tile([128, 2 * HIDDEN], F32, tag="pa_mm")
            pr = pr_f[:, :D0]
            nc.tensor.matmul(
                out=pr, lhsT=xT[:], rhs=wcat_s[0][:, D0:], start=True, stop=True
            )
            nc.vector.tensor_tensor(
                out=xr_sb[0][:, b, :D0], in0=pr, in1=biasr_s[0][:], op=A.add
            )

        # barrier proxy: one op depending on all tab0 writes; gathers dep on it
        barrier0 = nc.scalar.copy(out=ident[:1, :1], in_=ident[:1, :1])
        for wi in tab0_writes:
            add_dep_helper(barrier0.ins, wi.ins, sync=True, reason="tab0 done")

        # ============ layers
        ag_calls = {0: [barrier0]}  # per-layer list of deps for gathers

        for l, (Din, D, H, C, use_elu) in enumerate(LAYERS):
            HD = H + D
            xr_cur = xr_sb[l % 2]
            xr_nxt = xr_sb[(l + 1) % 2]
            gather_deps = ag_calls[l]
            if l < 2:
                Dn = LAYERS[l + 1][1]
                pa_writesA, pa_writesB = [], []
            if l == 2:
                pool_ps = psum_pool.tile([G, OUT_CH], F32, tag="pool")

            state = {}

            def stage1(b, l=l, Din=Din, D=D, H=H, C=C, state=state,
                       xr_cur=xr_cur, gather_deps=gather_deps):
                dcol = s1.tile([128, TB, 1], F32, tag="dcol")
                nc.sync.dma_start(
                    out=dcol[:], in_=dst_col[b * 128 : (b + 1) * 128, :]
                )
                idxt = s1.tile([128, TB], I32, tag="idxt")
                nc.sync.dma_start(
                    out=idxt[:], in_=src_idx[b * 128 : (b + 1) * 128, :]
                )
                drep = s1b.tile([128, TB * 128], BF, tag="drep")
                nc.sync.dma_start(
                    out=drep[:],
                    in_=dst_row[b : b + 1, :].to_broadcast([128, TB * 128]),
                )
                # mT[q, (t,s)] = (dstrow(edge t*128+s) == q): per-partition scalar cmp
                mT = s1b.tile([128, TB, 128], BF, tag="mT")
                nc.vector.tensor_scalar(
                    out=mT[:].rearrange("p t q -> p (t q)"), in0=drep[:],
                    scalar1=iota_c[:, :1], scalar2=None, op0=A.is_equal)
                # mE[p, t, j] = (dcol[p,t] == j): per-tile, iota dense in0
                mE = s1.tile([128, TB, 128], BF, tag="mE")
                for t in range(TB):
                    nc.vector.tensor_scalar(
                        out=mE[:, t, :], in0=iota_sm[:],
                        scalar1=dcol[:, t, :1], scalar2=None, op0=A.is_equal)
                v_all = s1.tile([128, TB, D], BF, tag="v")
                for t in range(TB):
                    ex = psum_exp.tile([128, D], F32, tag="exp")
                    nc.tensor.matmul(
                        out=ex[:], lhsT=mT[:, t, :], rhs=xr_cur[:, b, :D],
                        start=True, stop=True)
                    nc.scalar.copy(out=v_all[:, t, :], in_=ex[:])
                g_all = s1.tile([128, TB, D], BF, tag="g")
                for t in range(TB):
                    g = nc.gpsimd.indirect_dma_start(
                        out=g_all[:, t, :],
                        out_offset=None,
                        in_=tabs[l][:],
                        in_offset=bass.IndirectOffsetOnAxis(ap=idxt[:, t : t + 1], axis=0),
                        compute_op=A.bypass,
                    )
                    for dep in gather_deps:
                        add_dep_helper(g.ins, dep.ins, sync=True, reason="gather after table ready")
                state[b] = (v_all, g_all, mE)

            def stage2(b, l=l, Din=Din, D=D, H=H, C=C, HD=HD, state=state,
                       xr_cur=xr_cur, xr_nxt=xr_nxt, use_elu=use_elu):
                v_all, g_all, mE = state.pop(b)
                vs = s2.tile([128, TB * D], BF, tag="vsum")
                nc.vector.tensor_tensor(
                    out=vs[:], in0=g_all[:].rearrange("p t d -> p (t d)"),
                    in1=v_all[:].rearrange("p t d -> p (t d)"), op=A.add)
                v4 = g_all[:].rearrange("p t (h c) -> p t h c", h=H)
                l_all = s2.tile([128, TB * D], BF, tag="lrelu")
                vf = vs[:]
                nc.vector.scalar_tensor_tensor(
                    out=l_all[:], in0=vf, scalar=NEG_SLOPE, in1=vf,
                    op0=A.mult, op1=A.max)
                p_all = s2.tile([128, TB, H, C], BF, tag="patt")
                nc.vector.tensor_tensor(
                    out=p_all[:].rearrange("p t h c -> p (t h c)"), in0=l_all[:],
                    in1=att_s[l][:], op=A.mult)
                lg = s2.tile([128, TB, H], F32, tag="lg")
                nc.vector.tensor_reduce(
                    out=lg[:], in_=p_all[:], axis=mybir.AxisListType.X, op=A.add)
                e_t = s2.tile([128, TB, H, 1], BF, tag="expv")
                nc.scalar.activation(out=e_t[:], in_=lg[:], func=ACTF.Exp)
                w_all = s2.tile([128, TB, HD], BF, tag="wall")
                nc.scalar.copy(out=w_all[:, :, :H], in_=e_t[:])
                nc.vector.tensor_tensor(
                    out=w_all[:, :, H:].rearrange("p t (h c) -> p t h c", h=H),
                    in0=v4,
                    in1=e_t[:].to_broadcast([128, TB, H, C]), op=A.mult)

                o_ps = psum_agg.tile([128, HD], F32, tag="agg")
                for t in range(TB):
                    nc.tensor.matmul(
                        out=o_ps[:], lhsT=mE[:, t, :], rhs=w_all[:, t, :],
                        start=(t == 0), stop=(t == TB - 1))

                dn = nodep.tile([128, H], F32, tag="dn")
                nc.vector.tensor_scalar(
                    out=dn[:], in0=o_ps[:, :H], scalar1=1e-30, scalar2=None, op0=A.add)
                rc = nodep.tile([128, H], F32, tag="rc")
                nc.vector.reciprocal(out=rc[:], in_=dn[:])
                onorm = nodep.tile([128, H, C], F32, tag="onorm")
                nc.vector.tensor_tensor(
                    out=onorm[:],
                    in0=o_ps[:, H:].rearrange("p (h c) -> p h c", h=H),
                    in1=rc[:].rearrange("p (h o) -> p h o", h=H).to_broadcast([128, H, C]),
                    op=A.mult)
                hb = nodep.tile([128, D], F32, tag="hb")
                nc.vector.tensor_tensor(
                    out=hb[:], in0=onorm[:].rearrange("p h c -> p (h c)"),
                    in1=biasout_s[l][:], op=A.add)
                if use_elu:
                    amax = nodep.tile([128, D], F32, tag="amax")
                    nc.vector.tensor_scalar(
                        out=amax[:], in0=hb[:], scalar1=0.0, scalar2=None, op0=A.max)
                    amin = nodep.tile([128, D], F32, tag="amin")
                    nc.vector.tensor_scalar(
                        out=amin[:], in0=hb[:], scalar1=0.0, scalar2=None, op0=A.min)
                    aexp = nodep.tile([128, D], F32, tag="aexp")
                    nc.scalar.activation(out=aexp[:], in_=amin[:], func=ACTF.Exp)
                    h_t = nodep.tile([128, D], F32, tag="h")
                    nc.vector.scalar_tensor_tensor(
                        out=h_t[:], in0=amax[:], scalar=-1.0, in1=aexp[:],
                        op0=A.add, op1=A.add)
                    # fused projection for layer l+1
                    Dn = LAYERS[l + 1][1]
                    tp = psum_pat.tile([128, 128], F32, tag="pa_tr")
                    nc.tensor.transpose(out=tp[:], in_=h_t[:], identity=ident[:])
                    hT = nodep.tile([128, 128], BF, tag="pa_hT")
                    nc.scalar.copy(out=hT[:], in_=tp[:])
                    pp_f = psum_pam.tile([128, 2 * HIDDEN], F32, tag="pa_mm")
                    pp = pp_f[:, : 2 * Dn]
                    nc.tensor.matmul(
                        out=pp, lhsT=hT[:], rhs=wcat_s[l + 1][:],
                        start=True, stop=True)
                    xl_t = nodep.tile([128, Dn], F8, tag="pa_xl")
                    nc.scalar.copy(out=xl_t[:], in_=pp[:, :Dn])
                    if b < CHA:
                        wi = nc.sync.dma_start(
                            out=ccA[l + 1][b * 128 : (b + 1) * 128, :], in_=xl_t[:])
                        pa_writesA.append(wi)
                    else:
                        wi = nc.sync.dma_start(
                            out=ccB[l + 1][(b - CHA) * 128 : (b - CHA + 1) * 128, :],
                            in_=xl_t[:])
                        pa_writesB.append(wi)
                    nc.vector.tensor_tensor(
                        out=xr_nxt[:, b, :Dn], in0=pp[:, Dn:], in1=biasr_s[l + 1][:],
                        op=A.add)
                    if b == CHA - 1:
                        agA = nc.gpsimd.collective_compute(
                            "AllGather", A.bypass,
                            replica_groups=[list(range(NCORES))],
                            ins=[ccA[l + 1][:]],
                            outs=[tabs[l + 1][0 : NCORES * CHA * 128, :]],
                        )
                        for wi in pa_writesA:
                            add_dep_helper(agA.ins, wi.ins, sync=True, reason="agA")
                        ag_calls[l + 1] = [agA]
                    if b == NBLK - 1:
                        agB = nc.gpsimd.collective_compute(
                            "AllGather", A.bypass,
                            replica_groups=[list(range(NCORES))],
                            ins=[ccB[l + 1][:]],
                            outs=[tabs[l + 1][NCORES * CHA * 128 :, :]],
                        )
                        for wi in pa_writesB:
                            add_dep_helper(agB.ins, wi.ins, sync=True, reason="agB")
                        ag_calls[l + 1].append(agB)
                else:
                    h2b = nodep.tile([128, D], BF, tag="h2b")
                    nc.vector.tensor_scalar(
                        out=h2b[:], in0=hb[:], scalar1=0.0, scalar2=None, op0=A.add)
                    pm_t = nodep.tile([128, G], BF, tag="pmt")
                    nc.sync.dma_start(
                        out=pm_t[:], in_=pool_mask[b * 128 : (b + 1) * 128, :])
                    nc.tensor.matmul(
                        out=pool_ps[:], lhsT=pm_t[:], rhs=h2b[:],
                        start=(b == 0), stop=(b == NBLK - 1))

            for b in range(NBLK + LAG):
                if b < NBLK:
                    stage1(b)
                if b >= LAG:
                    stage2(b - LAG)

        pool_sb = nodep.tile([G, OUT_CH], F32, tag="poolsb")
        nc.scalar.copy(out=pool_sb[:], in_=pool_ps[:])
        nc.sync.dma_start(out=pool_out[:], in_=pool_sb[:])

    nc.compile()
    return nc


# ---------------------------------------------------------------- runner
_BUILD_CACHE = {}


def run(cfg, inp, trace=False):
    from concourse import bass_utils

    maps, counts = prep(cfg, np.asarray(inp["x"], np.float32), inp["edge_index"], inp["batch"])
    w = prep_weights(cfg, inp)
    for m in maps:
        m.update(w)

    key = (cfg["N"], cfg["G"], cfg["TB"], cfg["IN_CH"])
    if key not in _BUILD_CACHE:
        _BUILD_CACHE[key] = build(cfg)
    nc = _BUILD_CACHE[key]

    res = bass_utils.run_bass_kernel_spmd(
        nc, maps, core_ids=list(range(NCORES)), trace=trace
    )
    total = np.zeros((cfg["G"], OUT_CH), np.float64)
    for k in range(NCORES):
        total += res.results[k]["pool_out"].astype(np.float64)
    out = (total / np.maximum(counts, 1.0)[:, None]).astype(np.float32)
    return out, res


def kernel(**inputs) -> np.ndarray:
    cfg = make_cfg()
    out, _ = run(cfg, inputs, trace=False)
    return out



# revision 21
# speedup vs baseline: 1.0872x; 1.0872x over previous
"""GATv2 3-layer GNN on 8 Trainium2 NeuronCores (Bass/Tile) — v5.

Key structure (per core):
  - Nodes are host-binned into 8*49=392 blocks of 128 slots with balanced
    in-degree per block.
  - The xl gather uses gpsimd dma_gather (one instruction per table-half
    per block, ~1us SWDGE each) instead of per-tile indirect DMAs
    (994ns fixed overhead each, 16 per block).  dma_gather indices are
    int16, so the 50176-row table is split in two halves; each block's
    edges are packed half-0-first into whole 128-slot tiles, padded with
    dummy row-0 gathers so every core runs identical shapes (SPMD).
  - Tables are bf16 (dma_gather needs 256B-multiple rows; also improves
    accuracy over fp8).  Layer 2 (D=64) pads table rows to 128 cols.
  - Layer 0: every core builds the FULL xl0 table locally from a
    pre-transposed bf16 copy of x (no AllGather for layer 0).
  - Layers 1,2: xl shards are exchanged with a 2-chunk AllGather
    (blocks 0..CHA-1 early, rest late) so most of the exchange hides
    behind phase-B work of the producing layer.
  - Phase B is software-pipelined: stage1 (mask build, xr-expand
    matmuls batched into one PSUM tile + one copy per half, dma_gather)
    runs LAG blocks ahead of stage2 (edge math, one-hot aggregation,
    node update + fused projection of the NEXT layer).
  - bias trick: the table holds x@Wl WITHOUT bias; bl is folded into
    xr's bias (v = xl'+xr' is unchanged) and into the output bias.
"""

import sys

if "/opt/trn_rl_repo" not in sys.path:
    sys.path.insert(0, "/opt/trn_rl_repo")

import numpy as np
import ml_dtypes

BF16 = ml_dtypes.bfloat16

NEG_SLOPE = 0.2
N_NODES = 50000
N_EDGES = 800000
N_GRAPHS = 64
IN_CH = 128
HIDDEN = 128
HEADS = 4
OUT_CH = 64
NCORES = 8


def make_cfg(n_nodes=N_NODES, n_graphs=N_GRAPHS, in_ch=IN_CH):
    npc = n_nodes // NCORES
    assert npc * NCORES == n_nodes
    nblk = (npc + 127) // 128
    np_pad = nblk * 128
    cha = max(1, (nblk * 3) // 5)  # early AG chunk (blocks [0, cha))
    trows = NCORES * np_pad
    return dict(
        N=n_nodes,
        G=n_graphs,
        NPC=npc,
        NP=np_pad,
        NBLK=nblk,
        CHA=cha,
        CHB=nblk - cha,
        GBLK=NCORES * nblk,
        TROWS=trows,
        HML=trows // 2,
        IN_CH=in_ch,
        T0=None,  # per-block tile counts for table half 0 (list, len NBLK)
        T1=None,
        LAYERS=[
            (in_ch, HIDDEN, HEADS, HIDDEN // HEADS, True),
            (HIDDEN, HIDDEN, HEADS, HIDDEN // HEADS, True),
            (HIDDEN, OUT_CH, 1, OUT_CH, False),
        ],
    )


# ---------------------------------------------------------------- host prep
def _balanced_bins(deg, nbins, binsz):
    """Assign nodes to bins (each bin holds exactly binsz nodes) minimizing
    max total degree per bin.  Greedy: degree-desc, min-load non-full bin.
    Returns slot_of[node] = bin*binsz + position."""
    import heapq

    n = deg.shape[0]
    order = np.argsort(-deg, kind="stable")
    heap = [(0, b) for b in range(nbins)]
    heapq.heapify(heap)
    fill = np.zeros(nbins, np.int64)
    load = np.zeros(nbins, np.int64)
    slot_of = np.empty(n, np.int64)
    for nd in order:
        while True:
            l, b = heapq.heappop(heap)
            if fill[b] < binsz:
                break
        slot_of[nd] = b * binsz + fill[b]
        fill[b] += 1
        load[b] += deg[nd]
        if fill[b] < binsz:
            heapq.heappush(heap, (load[b], b))
    return slot_of, int(load.max())


def tabrow_of_slot(cfg, slot):
    """Map global slot id -> table row (2-chunk AllGather layout)."""
    NP, NBLK, CHA = cfg["NP"], cfg["NBLK"], cfg["CHA"]
    c = slot // NP
    loc = slot % NP
    b = loc // 128
    r = loc % 128
    rowsA = NCORES * CHA * 128
    return np.where(
        b < CHA,
        c * CHA * 128 + b * 128 + r,
        rowsA + c * (NBLK - CHA) * 128 + (b - CHA) * 128 + r,
    )


def _wrap16(lst):
    """dma_gather index layout: idx k -> [k%16, k//16], replicated x8."""
    n = lst.shape[0]
    assert n % 16 == 0
    w = lst.reshape(n // 16, 16).T  # [16, W]
    return np.tile(w, (8, 1)).astype(np.int16)  # [128, W]


def prep(cfg, x, edge_index, batch):
    NPC, NP, NBLK, G, CHA = cfg["NPC"], cfg["NP"], cfg["NBLK"], cfg["G"], cfg["CHA"]
    GBLK, HML = cfg["GBLK"], cfg["HML"]
    Din = cfg["IN_CH"]
    src = np.asarray(edge_index[0], dtype=np.int64)
    dst = np.asarray(edge_index[1], dtype=np.int64)
    batch = np.asarray(batch, dtype=np.int64)
    x = np.asarray(x, dtype=np.float32)
    N = x.shape[0]

    deg = np.bincount(dst, minlength=N)
    slot_of, maxload = _balanced_bins(deg, GBLK, 128)

    node_of_slot = np.full(GBLK * 128, -1, np.int64)
    node_of_slot[slot_of] = np.arange(N)

    # permuted x, laid out in TABLE-ROW block order, transposed per block
    x_slot = np.zeros((GBLK * 128, Din), np.float32)
    valid = node_of_slot >= 0
    x_slot[valid] = x[node_of_slot[valid]]
    tabrow = np.asarray(tabrow_of_slot(cfg, np.arange(GBLK * 128)))
    x_tab = np.zeros_like(x_slot)
    x_tab[tabrow] = x_slot
    assert GBLK % 8 == 0
    xfullT = (
        x_tab.reshape(GBLK // 8, 8, 128, Din)
        .transpose(0, 3, 1, 2)
        .reshape((GBLK // 8) * Din, 8 * 128)
    ).astype(BF16)

    # edges
    sd = slot_of[dst]
    ss = slot_of[src]
    trow = np.asarray(tabrow_of_slot(cfg, ss))
    core_of = sd // NP
    dloc = sd % NP
    bloc = dloc // 128
    drow = dloc % 128

    # ---- pass 1: per (core, block) edge lists split by table half
    half = (trow >= HML).astype(np.int64)
    key = (core_of * NBLK + bloc) * 2 + half
    order = np.argsort(key, kind="stable")
    ks = key[order]
    tr_s = trow[order]
    ed_s = drow[order]
    bounds = np.searchsorted(ks, np.arange(NCORES * NBLK * 2 + 1))
    ed_rows = [[None] * NBLK for _ in range(NCORES)]  # (rows0, d0, rows1, d1)
    for c in range(NCORES):
        for b in range(NBLK):
            k0i = (c * NBLK + b) * 2
            s0, e0 = bounds[k0i], bounds[k0i + 1]
            s1_, e1_ = bounds[k0i + 1], bounds[k0i + 2]
            ed_rows[c][b] = (
                tr_s[s0:e0], ed_s[s0:e0], tr_s[s1_:e1_] - HML, ed_s[s1_:e1_])

    # ---- pass 2: shared per-block tile counts (max over cores)
    T0 = [0] * NBLK
    T1 = [0] * NBLK
    for b in range(NBLK):
        for c in range(NCORES):
            r0, _, r1, _ = ed_rows[c][b]
            T0[b] = max(T0[b], (len(r0) + 127) // 128)
            T1[b] = max(T1[b], (len(r1) + 127) // 128)
    T0 = [max(t, 1) for t in T0]
    T1 = [max(t, 1) for t in T1]
    assert max(max(T0), max(T1)) <= 16, (max(T0), max(T1))
    cfg["T0"], cfg["T1"] = T0, T1
    TMAXH0 = max(T0)
    TMAXH1 = max(T1)
    TMAX = max(a + b for a, b in zip(T0, T1))
    cfg["TMAXH0"], cfg["TMAXH1"], cfg["TMAX"] = TMAXH0, TMAXH1, TMAX
    W0, W1 = TMAXH0 * 8, TMAXH1 * 8

    # ---- pass 3: per-core arrays
    maps = []
    for c in range(NCORES):
        idxg = np.zeros((NBLK * 128, W0 + W1), np.int16)
        dst_col = np.full((NBLK * 128, TMAX), -1.0, np.float32)
        dst_row = np.full((NBLK, TMAX * 128), -1.0, np.float32)
        for b in range(NBLK):
            r0, d0, r1, d1 = ed_rows[c][b]
            t0, t1 = T0[b], T1[b]
            L0 = np.zeros(t0 * 128, np.int64)
            L0[: len(r0)] = r0
            L1 = np.zeros(t1 * 128, np.int64)
            L1[: len(r1)] = r1
            idxg[b * 128 : (b + 1) * 128, : t0 * 8] = _wrap16(L0)
            idxg[b * 128 : (b + 1) * 128, W0 : W0 + t1 * 8] = _wrap16(L1)
            dv = np.full((t0 + t1) * 128, -1.0, np.float32)
            dv[: len(d0)] = d0
            dv[t0 * 128 : t0 * 128 + len(d1)] = d1
            dst_row[b, : (t0 + t1) * 128] = dv
            dc = dv.reshape(t0 + t1, 128).T  # [128, T]
            dst_col[b * 128 : (b + 1) * 128, : t0 + t1] = dc

        # own x^T blocks (for the xr projection pass), in own-block order
        own_tabrows = np.asarray(tabrow_of_slot(cfg, c * NP + np.arange(NP)))
        xownT = (
            x_tab[own_tabrows]
            .reshape(NBLK, 128, Din)
            .transpose(0, 2, 1)
            .reshape(NBLK * Din, 128)
        ).astype(BF16)

        # pool mask [NP, G] over own slots
        pm = np.zeros((NP, G), np.float32)
        own_nodes = node_of_slot[c * NP : (c + 1) * NP]
        vv = own_nodes >= 0
        pm[np.arange(NP)[vv], batch[own_nodes[vv]]] = 1.0

        maps.append(
            dict(
                xfullT=xfullT,
                xownT=xownT,
                idxg=idxg,
                dst_col=dst_col.astype(BF16),
                dst_row=dst_row.astype(BF16),
                pool_mask=pm.astype(BF16),
            )
        )

    counts = np.bincount(batch, minlength=G).astype(np.float32)
    return maps, counts


def prep_weights(cfg, inp):
    w = {}
    for l in range(3):
        Wl = np.asarray(inp[f"Wl{l}"], np.float32)
        bl = np.asarray(inp[f"bl{l}"], np.float32)
        Wr = np.asarray(inp[f"Wr{l}"], np.float32)
        br = np.asarray(inp[f"br{l}"], np.float32)
        bo = np.asarray(inp[f"bias{l}"], np.float32)
        D = Wl.shape[1]
        # table holds x@Wl (no bias); xr bias = bl+br; out bias += bl
        w[f"wcat{l}"] = np.concatenate([Wl, Wr], axis=1).astype(BF16)  # [Din,2D]
        w[f"bias_r{l}"] = np.broadcast_to((bl + br)[None, :], (128, D)).copy()
        w[f"bias_out{l}"] = np.broadcast_to((bo + bl)[None, :], (128, D)).copy()
    w["iota_col"] = np.arange(128, dtype=np.float32)[:, None]
    TMAX = cfg["TMAX"]
    w["iota_rep"] = np.broadcast_to(
        np.arange(128, dtype=np.float32)[None, None, :], (128, TMAX, 128)
    ).reshape(128, TMAX * 128).astype(BF16)
    for l in range(3):
        D = [HIDDEN, HIDDEN, OUT_CH][l]
        w[f"att{l}r"] = np.broadcast_to(
            np.asarray(inp[f"att{l}"], np.float32).reshape(1, 1, D), (128, TMAX, D)
        ).reshape(128, TMAX * D).astype(BF16)
    w["ident"] = np.eye(128, dtype=np.float32)
    return w


# ---------------------------------------------------------------- device build
def build(cfg):
    from concourse import bass, bacc, mybir
    import concourse.tile as tile
    from concourse.tile import add_dep_helper

    F32 = mybir.dt.float32
    BF = mybir.dt.bfloat16
    I16 = mybir.dt.int16
    A = mybir.AluOpType
    ACTF = mybir.ActivationFunctionType

    NP, NBLK, TROWS, G = cfg["NP"], cfg["NBLK"], cfg["TROWS"], cfg["G"]
    CHA, CHB, GBLK, HML = cfg["CHA"], cfg["CHB"], cfg["GBLK"], cfg["HML"]
    T0, T1, TMAX = cfg["T0"], cfg["T1"], cfg["TMAX"]
    TMAXH0, TMAXH1 = cfg["TMAXH0"], cfg["TMAXH1"]
    W0, W1 = TMAXH0 * 8, TMAXH1 * 8
    Din0 = cfg["IN_CH"]
    LAYERS = cfg["LAYERS"]
    LAG = 2

    nc = bacc.Bacc(
        "TRN2",
        target_bir_lowering=False,
        debug=False,
        enable_asserts=False,
        num_devices=NCORES,
    )

    ext = {}

    def ein(name, shape, dt):
        ext[name] = nc.dram_tensor(name, shape, dt, kind="ExternalInput").ap()
        return ext[name]

    xfullT = ein("xfullT", [(GBLK // 8) * Din0, 8 * 128], BF)
    xownT = ein("xownT", [NBLK * Din0, 128], BF)
    idxg_d = ein("idxg", [NBLK * 128, W0 + W1], I16)
    dst_col = ein("dst_col", [NBLK * 128, TMAX], BF)
    dst_row = ein("dst_row", [NBLK, TMAX * 128], BF)
    pool_mask = ein("pool_mask", [NP, G], BF)
    iota_col_d = ein("iota_col", [128, 1], F32)
    iota_rep_d = ein("iota_rep", [128, TMAX * 128], BF)
    ident_d = ein("ident", [128, 128], F32)
    wcat_d, biasr_d, att_d, biasout_d = [], [], [], []
    for l, (Din, D, H, C, _) in enumerate(LAYERS):
        wcat_d.append(ein(f"wcat{l}", [Din, 2 * D], BF))
        biasr_d.append(ein(f"bias_r{l}", [128, D], F32))
        att_d.append(ein(f"att{l}r", [128, TMAX * D], BF))
        biasout_d.append(ein(f"bias_out{l}", [128, D], F32))

    pool_out = nc.dram_tensor("pool_out", [G, OUT_CH], F32, kind="ExternalOutput").ap()

    # internal DRAM: tables are bf16, 128 cols even for layer 2 (gather rows
    # must be 256B multiples)
    tabs = []
    ccA, ccB = [None] * 3, [None] * 3
    for l in range(3):
        tabs.append(
            nc.dram_tensor(
                f"tab{l}", [TROWS, 128], BF, kind="Internal", addr_space="Shared"
            ).ap()
        )
        if l >= 1:
            ccA[l] = nc.dram_tensor(f"ccA{l}", [CHA * 128, 128], BF, kind="Internal").ap()
            ccB[l] = nc.dram_tensor(f"ccB{l}", [CHB * 128, 128], BF, kind="Internal").ap()

    from contextlib import ExitStack

    with tile.TileContext(nc) as tc, ExitStack() as pools:
        const = pools.enter_context(tc.tile_pool(name="const", bufs=1))
        s1 = pools.enter_context(tc.tile_pool(name="s1", bufs=LAG + 2))
        s1b = pools.enter_context(tc.tile_pool(name="s1b", bufs=2))
        s2 = pools.enter_context(tc.tile_pool(name="s2", bufs=2))
        nodep = pools.enter_context(tc.tile_pool(name="nodep", bufs=2))
        # PSUM: 8 banks x 2KB.  vexp 4 banks, agg 1, pam 1, pat 1, pool 1.
        psum_exp = pools.enter_context(tc.tile_pool(name="psum_exp", bufs=1, space="PSUM"))
        psum_agg = pools.enter_context(tc.tile_pool(name="psum_agg", bufs=1, space="PSUM"))
        psum_pam = pools.enter_context(tc.tile_pool(name="psum_pam", bufs=1, space="PSUM"))
        psum_pat = pools.enter_context(tc.tile_pool(name="psum_pat", bufs=1, space="PSUM"))
        psum_pool = pools.enter_context(tc.tile_pool(name="psum_pool", bufs=1, space="PSUM"))

        # persistent SBUF: xr tables (double-buffered across layers)
        xr_sb = [
            nc.alloc_sbuf_tensor(f"xr_sb{k}", [128, NBLK, HIDDEN], BF).ap()
            for k in range(2)
        ]

        def const_tile(shape, dt, src_ap, tag):
            t = const.tile(shape, dt, tag=tag)
            nc.sync.dma_start(out=t[:], in_=src_ap)
            return t

        ident = const_tile([128, 128], F32, ident_d[:], "ident")
        iota_c = const_tile([128, 1], F32, iota_col_d[:], "iotac")
        iota_rep = const_tile(
            [128, TMAX, 128], BF,
            iota_rep_d[:].rearrange("p (t q) -> p t q", t=TMAX), "iotarep")
        wcat_s, biasr_s, att_s, biasout_s = [], [], [], []
        for l, (Din, D, H, C, _) in enumerate(LAYERS):
            wcat_s.append(const_tile([Din, 2 * D], BF, wcat_d[l][:], f"wc{l}"))
            biasr_s.append(const_tile([128, D], F32, biasr_d[l][:], f"br{l}"))
            att_s.append(const_tile([128, TMAX * D], BF, att_d[l][:], f"at{l}"))
            biasout_s.append(const_tile([128, D], F32, biasout_d[l][:], f"bo{l}"))

        # ============ layer 0: local full-table build + own xr pass
        D0 = LAYERS[0][1]
        tab0_writes = []
        GRP = 8
        assert GBLK % GRP == 0
        for gg in range(GBLK // GRP):
            xT8 = nodep.tile([Din0, GRP, 128], BF, tag="t0_xT")
            nc.sync.dma_start(
                out=xT8[:],
                in_=xfullT[gg * Din0 : (gg + 1) * Din0, :],
            )
            vps8 = psum_exp.tile([128, GRP, D0], F32, tag="vexp")
            for k in range(GRP):
                nc.tensor.matmul(
                    out=vps8[:, k, :], lhsT=xT8[:, k, :], rhs=wcat_s[0][:, :D0],
                    start=True, stop=True
                )
            xl8 = nodep.tile([128, GRP, D0], BF, tag="t0_xl")
            nc.scalar.copy(out=xl8[:], in_=vps8[:])
            wi = nc.sync.dma_start(
                out=tabs[0][gg * GRP * 128 : (gg + 1) * GRP * 128, :].rearrange(
                    "(g p) d -> p g d", g=GRP
                ),
                in_=xl8[:],
            )
            tab0_writes.append(wi)

        for b in range(NBLK):
            xT = nodep.tile([Din0, 128], BF, tag="own_xT")
            nc.sync.dma_start(out=xT[:], in_=xownT[b * Din0 : (b + 1) * Din0, :])
            pr_f = psum_pam.tile([128, 2 * HIDDEN], F32, tag="pa_mm")
            pr = pr_f[:, :D0]
            nc.tensor.matmul(
                out=pr, lhsT=xT[:], rhs=wcat_s[0][:, D0:], start=True, stop=True
            )
            nc.vector.tensor_tensor(
                out=xr_sb[0][:, b, :D0], in0=pr, in1=biasr_s[0][:], op=A.add
            )

        # barrier proxy: one op depending on all tab0 writes; gathers dep on it
        barrier0 = nc.scalar.copy(out=ident[:1, :1], in_=ident[:1, :1])
        for wi in tab0_writes:
            add_dep_helper(barrier0.ins, wi.ins, sync=True, reason="tab0 done")

        # ============ layers
        ag_calls = {0: [barrier0]}  # per-layer list of deps for gathers

        for l, (Din, D, H, C, use_elu) in enumerate(LAYERS):
            HD = H + D
            xr_cur = xr_sb[l % 2]
            xr_nxt = xr_sb[(l + 1) % 2]
            gather_deps = ag_calls[l]
            if l < 2:
                pa_writesA, pa_writesB = [], []
            if l == 2:
                pool_ps = psum_pool.tile([G, OUT_CH], F32, tag="pool")

            state = {}

            def stage1(b, l=l, D=D, state=state,
                       xr_cur=xr_cur, gather_deps=gather_deps):
                t0, t1 = T0[b], T1[b]
                Tb = t0 + t1
                dcol = s1.tile([128, Tb, 1], BF, tag="dcol")
                nc.sync.dma_start(
                    out=dcol[:], in_=dst_col[b * 128 : (b + 1) * 128, :Tb]
                )
                idxt = s1.tile([128, W0 + W1], I16, tag="idxt")
                nc.sync.dma_start(
                    out=idxt[:], in_=idxg_d[b * 128 : (b + 1) * 128, :]
                )
                drep = s1b.tile([128, Tb * 128], BF, tag="drep")
                nc.sync.dma_start(
                    out=drep[:],
                    in_=dst_row[b : b + 1, : Tb * 128].to_broadcast([128, Tb * 128]),
                )
                # mT[q, (t,s)] = (dstrow(edge t*128+s) == q)
                mT = s1b.tile([128, Tb, 128], BF, tag="mT")
                nc.vector.tensor_scalar(
                    out=mT[:].rearrange("p t q -> p (t q)"), in0=drep[:],
                    scalar1=iota_c[:, :1], scalar2=None, op0=A.is_equal)
                # mE[p, t, j] = (dcol[p,t] == j)
                mE = s1.tile([128, Tb, 128], BF, tag="mE")
                nc.vector.tensor_tensor(
                    out=mE[:], in0=iota_rep[:, :Tb, :],
                    in1=dcol[:].to_broadcast([128, Tb, 128]), op=A.is_equal)
                # xr expansion per half into one PSUM tile + one copy
                v_all = s1.tile([128, Tb, D], BF, tag="v")
                for h, (tb, toff) in enumerate([(t0, 0), (t1, t0)]):
                    vps = psum_exp.tile([128, max(TMAXH0, TMAXH1, GRP), D0], F32,
                                        tag="vexp")
                    for t in range(tb):
                        nc.tensor.matmul(
                            out=vps[:, t, :D], lhsT=mT[:, toff + t, :],
                            rhs=xr_cur[:, b, :D], start=True, stop=True)
                    nc.scalar.copy(
                        out=v_all[:, toff : toff + tb, :], in_=vps[:, :tb, :D])
                # batched gathers: dma_gather per table half, chunked to <=8
                # tiles (1024 descs) -- the SWDGE ring holds 1024 descriptors
                g_all = s1.tile([128, Tb, 128], BF, tag="g")
                for tb_h, toff, ioff, roff in (
                    (t0, 0, 0, 0), (t1, t0, W0, HML)):
                    done = 0
                    while done < tb_h:
                        ch = min(8, tb_h - done)
                        gi = nc.gpsimd.dma_gather(
                            out_ap=g_all[:, toff + done : toff + done + ch, :],
                            in_ap=tabs[l][roff : roff + HML, :],
                            idxs_ap=idxt[:, ioff + done * 8 : ioff + (done + ch) * 8],
                            num_idxs=ch * 128, num_idxs_reg=ch * 128,
                            elem_size=128, queue_num=0)
                        for dep in gather_deps:
                            add_dep_helper(gi.ins, dep.ins, sync=True,
                                           reason="gather after table ready")
                        done += ch
                state[b] = (v_all, g_all, mE)

            def stage2(b, l=l, D=D, H=H, C=C, HD=HD, state=state,
                       xr_cur=xr_cur, xr_nxt=xr_nxt, use_elu=use_elu):
                t0, t1 = T0[b], T1[b]
                Tb = t0 + t1
                v_all, g_all, mE = state.pop(b)
                gD = g_all[:, :, :D]
                vs = s2.tile([128, Tb, D], BF, tag="vsum")
                nc.vector.tensor_tensor(
                    out=vs[:], in0=gD, in1=v_all[:], op=A.add)
                v4 = gD.rearrange("p t (h c) -> p t h c", h=H)
                vf = vs[:].rearrange("p t d -> p (t d)")
                l_all = s2.tile([128, Tb * D], BF, tag="lrelu")
                nc.vector.scalar_tensor_tensor(
                    out=l_all[:], in0=vf, scalar=NEG_SLOPE, in1=vf,
                    op0=A.mult, op1=A.max)
                p_all = s2.tile([128, Tb, H, C], BF, tag="patt")
                nc.vector.tensor_tensor(
                    out=p_all[:].rearrange("p t h c -> p (t h c)"), in0=l_all[:],
                    in1=att_s[l][:, : Tb * D], op=A.mult)
                lg = s2.tile([128, Tb, H], F32, tag="lg")
                nc.vector.tensor_reduce(
                    out=lg[:], in_=p_all[:], axis=mybir.AxisListType.X, op=A.add)
                e_t = s2.tile([128, Tb, H, 1], BF, tag="expv")
                nc.scalar.activation(out=e_t[:], in_=lg[:], func=ACTF.Exp)
                w_all = s2.tile([128, Tb, HD], BF, tag="wall")
                nc.scalar.copy(out=w_all[:, :, :H], in_=e_t[:])
                nc.vector.tensor_tensor(
                    out=w_all[:, :, H:].rearrange("p t (h c) -> p t h c", h=H),
                    in0=v4,
                    in1=e_t[:].to_broadcast([128, Tb, H, C]), op=A.mult)

                o_ps = psum_agg.tile([128, HD], F32, tag="agg")
                for t in range(Tb):
                    nc.tensor.matmul(
                        out=o_ps[:], lhsT=mE[:, t, :], rhs=w_all[:, t, :],
                        start=(t == 0), stop=(t == Tb - 1))

                dn = nodep.tile([128, H], F32, tag="dn")
                nc.vector.tensor_scalar(
                    out=dn[:], in0=o_ps[:, :H], scalar1=1e-30, scalar2=None, op0=A.add)
                rc = nodep.tile([128, H], F32, tag="rc")
                nc.vector.reciprocal(out=rc[:], in_=dn[:])
                onorm = nodep.tile([128, H, C], F32, tag="onorm")
                nc.vector.tensor_tensor(
                    out=onorm[:],
                    in0=o_ps[:, H:].rearrange("p (h c) -> p h c", h=H),
                    in1=rc[:].rearrange("p (h o) -> p h o", h=H).to_broadcast([128, H, C]),
                    op=A.mult)
                hb = nodep.tile([128, D], F32, tag="hb")
                nc.vector.tensor_tensor(
                    out=hb[:], in0=onorm[:].rearrange("p h c -> p (h c)"),
                    in1=biasout_s[l][:], op=A.add)
                if use_elu:
                    amax = nodep.tile([128, D], F32, tag="amax")
                    nc.vector.tensor_scalar(
                        out=amax[:], in0=hb[:], scalar1=0.0, scalar2=None, op0=A.max)
                    amin = nodep.tile([128, D], F32, tag="amin")
                    nc.vector.tensor_scalar(
                        out=amin[:], in0=hb[:], scalar1=0.0, scalar2=None, op0=A.min)
                    aexp = nodep.tile([128, D], F32, tag="aexp")
                    nc.scalar.activation(out=aexp[:], in_=amin[:], func=ACTF.Exp)
                    h_t = nodep.tile([128, D], F32, tag="h")
                    nc.vector.scalar_tensor_tensor(
                        out=h_t[:], in0=amax[:], scalar=-1.0, in1=aexp[:],
                        op0=A.add, op1=A.add)
                    # fused projection for layer l+1
                    Dn = LAYERS[l + 1][1]
                    tp = psum_pat.tile([128, 128], F32, tag="pa_tr")
                    nc.tensor.transpose(out=tp[:], in_=h_t[:], identity=ident[:])
                    hT = nodep.tile([128, 128], BF, tag="pa_hT")
                    nc.scalar.copy(out=hT[:], in_=tp[:])
                    pp_f = psum_pam.tile([128, 2 * HIDDEN], F32, tag="pa_mm")
                    pp = pp_f[:, : 2 * Dn]
                    nc.tensor.matmul(
                        out=pp, lhsT=hT[:], rhs=wcat_s[l + 1][:],
                        start=True, stop=True)
                    xl_t = nodep.tile([128, 128], BF, tag="pa_xl")
                    nc.scalar.copy(out=xl_t[:, :Dn], in_=pp[:, :Dn])
                    if Dn < 128:
                        nc.scalar.copy(out=xl_t[:, Dn:], in_=pp[:, :128 - Dn])
                    if b < CHA:
                        wi = nc.sync.dma_start(
                            out=ccA[l + 1][b * 128 : (b + 1) * 128, :], in_=xl_t[:])
                        pa_writesA.append(wi)
                    else:
                        wi = nc.sync.dma_start(
                            out=ccB[l + 1][(b - CHA) * 128 : (b - CHA + 1) * 128, :],
                            in_=xl_t[:])
                        pa_writesB.append(wi)
                    nc.vector.tensor_tensor(
                        out=xr_nxt[:, b, :Dn], in0=pp[:, Dn : 2 * Dn],
                        in1=biasr_s[l + 1][:], op=A.add)
                    if b == CHA - 1:
                        agA = nc.gpsimd.collective_compute(
                            "AllGather", A.bypass,
                            replica_groups=[list(range(NCORES))],
                            ins=[ccA[l + 1][:]],
                            outs=[tabs[l + 1][0 : NCORES * CHA * 128, :]],
                        )
                        for wi in pa_writesA:
                            add_dep_helper(agA.ins, wi.ins, sync=True, reason="agA")
                        ag_calls[l + 1] = [agA]
                    if b == NBLK - 1:
                        agB = nc.gpsimd.collective_compute(
                            "AllGather", A.bypass,
                            replica_groups=[list(range(NCORES))],
                            ins=[ccB[l + 1][:]],
                            outs=[tabs[l + 1][NCORES * CHA * 128 :, :]],
                        )
                        for wi in pa_writesB:
                            add_dep_helper(agB.ins, wi.ins, sync=True, reason="agB")
                        ag_calls[l + 1].append(agB)
                else:
                    h2b = nodep.tile([128, D], BF, tag="h2b")
                    nc.vector.tensor_scalar(
                        out=h2b[:], in0=hb[:], scalar1=0.0, scalar2=None, op0=A.add)
                    pm_t = nodep.tile([128, G], BF, tag="pmt")
                    nc.sync.dma_start(
                        out=pm_t[:], in_=pool_mask[b * 128 : (b + 1) * 128, :])
                    nc.tensor.matmul(
                        out=pool_ps[:], lhsT=pm_t[:], rhs=h2b[:],
                        start=(b == 0), stop=(b == NBLK - 1))

            for b in range(NBLK + LAG):
                if b < NBLK:
                    stage1(b)
                if b >= LAG:
                    stage2(b - LAG)

        pool_sb = nodep.tile([G, OUT_CH], F32, tag="poolsb")
        nc.scalar.copy(out=pool_sb[:], in_=pool_ps[:])
        nc.sync.dma_start(out=pool_out[:], in_=pool_sb[:])

    nc.compile()
    return nc


# ---------------------------------------------------------------- runner
_BUILD_CACHE = {}


def run(cfg, inp, trace=False):
    from concourse import bass_utils

    maps, counts = prep(cfg, np.asarray(inp["x"], np.float32), inp["edge_index"], inp["batch"])
    w = prep_weights(cfg, inp)
    for m in maps:
        m.update(w)

    key = (cfg["N"], cfg["G"], tuple(cfg["T0"]), tuple(cfg["T1"]))
    if key not in _BUILD_CACHE:
        _BUILD_CACHE[key] = build(cfg)
    nc = _BUILD_CACHE[key]

    res = bass_utils.run_bass_kernel_spmd(
        nc, maps, core_ids=list(range(NCORES)), trace=trace
    )
    total = np.zeros((cfg["G"], OUT_CH), np.float64)
    for k in range(NCORES):
        total += res.results[k]["pool_out"].astype(np.float64)
    out = (total / np.maximum(counts, 1.0)[:, None]).astype(np.float32)
    return out, res


def kernel(**inputs) -> np.ndarray:
    cfg = make_cfg()
    out, _ = run(cfg, inputs, trace=False)
    return out


# revision 25
# speedup vs baseline: 1.5805x; 1.4538x over previous
"""GATv2 3-layer GNN on 8 Trainium2 NeuronCores (Bass/Tile) — v5.

Key structure (per core):
  - Nodes are host-binned into 8*49=392 blocks of 128 slots with balanced
    in-degree per block.
  - The xl gather uses gpsimd dma_gather (one instruction per table-half
    per block, ~1us SWDGE each) instead of per-tile indirect DMAs
    (994ns fixed overhead each, 16 per block).  dma_gather indices are
    int16, so the 50176-row table is split in two halves; each block's
    edges are packed half-0-first into whole 128-slot tiles, padded with
    dummy row-0 gathers so every core runs identical shapes (SPMD).
  - Tables are bf16 (dma_gather needs 256B-multiple rows; also improves
    accuracy over fp8).  Layer 2 (D=64) pads table rows to 128 cols.
  - Layer 0: every core builds the FULL xl0 table locally from a
    pre-transposed bf16 copy of x (no AllGather for layer 0).
  - Layers 1,2: xl shards are exchanged with a 2-chunk AllGather
    (blocks 0..CHA-1 early, rest late) so most of the exchange hides
    behind phase-B work of the producing layer.
  - Phase B is software-pipelined: stage1 (mask build, xr-expand
    matmuls batched into one PSUM tile + one copy per half, dma_gather)
    runs LAG blocks ahead of stage2 (edge math, one-hot aggregation,
    node update + fused projection of the NEXT layer).
  - bias trick: the table holds x@Wl WITHOUT bias; bl is folded into
    xr's bias (v = xl'+xr' is unchanged) and into the output bias.
"""

import sys

if "/opt/trn_rl_repo" not in sys.path:
    sys.path.insert(0, "/opt/trn_rl_repo")

import numpy as np
import ml_dtypes

BF16 = ml_dtypes.bfloat16

NEG_SLOPE = 0.2
N_NODES = 50000
N_EDGES = 800000
N_GRAPHS = 64
IN_CH = 128
HIDDEN = 128
HEADS = 4
OUT_CH = 64
NCORES = 8


def make_cfg(n_nodes=N_NODES, n_graphs=N_GRAPHS, in_ch=IN_CH):
    npc = n_nodes // NCORES
    assert npc * NCORES == n_nodes
    nblk = (npc + 127) // 128
    np_pad = nblk * 128
    cha = max(1, (nblk * 3) // 5)  # early AG chunk (blocks [0, cha))
    trows = NCORES * np_pad
    return dict(
        N=n_nodes,
        G=n_graphs,
        NPC=npc,
        NP=np_pad,
        NBLK=nblk,
        CHA=cha,
        CHB=nblk - cha,
        GBLK=NCORES * nblk,
        TROWS=trows,
        HML=trows // 2,
        IN_CH=in_ch,
        T0=None,  # per-block tile counts for table half 0 (list, len NBLK)
        T1=None,
        LAYERS=[
            (in_ch, HIDDEN, HEADS, HIDDEN // HEADS, True),
            (HIDDEN, HIDDEN, HEADS, HIDDEN // HEADS, True),
            (HIDDEN, OUT_CH, 1, OUT_CH, False),
        ],
    )


# ---------------------------------------------------------------- host prep
def _balanced_bins(deg, nbins, binsz):
    """Assign nodes to bins (each bin holds exactly binsz nodes) minimizing
    max total degree per bin.  Greedy: degree-desc, min-load non-full bin.
    Returns slot_of[node] = bin*binsz + position."""
    import heapq

    n = deg.shape[0]
    order = np.argsort(-deg, kind="stable")
    heap = [(0, b) for b in range(nbins)]
    heapq.heapify(heap)
    fill = np.zeros(nbins, np.int64)
    load = np.zeros(nbins, np.int64)
    slot_of = np.empty(n, np.int64)
    for nd in order:
        while True:
            l, b = heapq.heappop(heap)
            if fill[b] < binsz:
                break
        slot_of[nd] = b * binsz + fill[b]
        fill[b] += 1
        load[b] += deg[nd]
        if fill[b] < binsz:
            heapq.heappush(heap, (load[b], b))
    return slot_of, int(load.max())


def tabrow_of_slot(cfg, slot):
    """Map global slot id -> table row (2-chunk AllGather layout)."""
    NP, NBLK, CHA = cfg["NP"], cfg["NBLK"], cfg["CHA"]
    c = slot // NP
    loc = slot % NP
    b = loc // 128
    r = loc % 128
    rowsA = NCORES * CHA * 128
    return np.where(
        b < CHA,
        c * CHA * 128 + b * 128 + r,
        rowsA + c * (NBLK - CHA) * 128 + (b - CHA) * 128 + r,
    )


def _wrap16(lst):
    """dma_gather index layout: idx k -> [k%16, k//16], replicated x8."""
    n = lst.shape[0]
    assert n % 16 == 0
    w = lst.reshape(n // 16, 16).T  # [16, W]
    return np.tile(w, (8, 1)).astype(np.int16)  # [128, W]


def prep(cfg, x, edge_index, batch):
    NPC, NP, NBLK, G, CHA = cfg["NPC"], cfg["NP"], cfg["NBLK"], cfg["G"], cfg["CHA"]
    GBLK, HML = cfg["GBLK"], cfg["HML"]
    Din = cfg["IN_CH"]
    src = np.asarray(edge_index[0], dtype=np.int64)
    dst = np.asarray(edge_index[1], dtype=np.int64)
    batch = np.asarray(batch, dtype=np.int64)
    x = np.asarray(x, dtype=np.float32)
    N = x.shape[0]

    deg = np.bincount(dst, minlength=N)
    slot_of, maxload = _balanced_bins(deg, GBLK, 128)

    node_of_slot = np.full(GBLK * 128, -1, np.int64)
    node_of_slot[slot_of] = np.arange(N)

    # permuted x, laid out in TABLE-ROW block order, transposed per block
    x_slot = np.zeros((GBLK * 128, Din), np.float32)
    valid = node_of_slot >= 0
    x_slot[valid] = x[node_of_slot[valid]]
    tabrow = np.asarray(tabrow_of_slot(cfg, np.arange(GBLK * 128)))
    x_tab = np.zeros_like(x_slot)
    x_tab[tabrow] = x_slot
    assert GBLK % 8 == 0
    xfullT = (
        x_tab.reshape(GBLK // 8, 8, 128, Din)
        .transpose(0, 3, 1, 2)
        .reshape((GBLK // 8) * Din, 8 * 128)
    ).astype(BF16)

    # edges
    sd = slot_of[dst]
    ss = slot_of[src]
    trow = np.asarray(tabrow_of_slot(cfg, ss))
    core_of = sd // NP
    dloc = sd % NP
    bloc = dloc // 128
    drow = dloc % 128

    # ---- pass 1: per (core, block) edge lists split by table half
    half = (trow >= HML).astype(np.int64)
    key = (core_of * NBLK + bloc) * 2 + half
    order = np.argsort(key, kind="stable")
    ks = key[order]
    tr_s = trow[order]
    ed_s = drow[order]
    bounds = np.searchsorted(ks, np.arange(NCORES * NBLK * 2 + 1))
    ed_rows = [[None] * NBLK for _ in range(NCORES)]  # (rows0, d0, rows1, d1)
    for c in range(NCORES):
        for b in range(NBLK):
            k0i = (c * NBLK + b) * 2
            s0, e0 = bounds[k0i], bounds[k0i + 1]
            s1_, e1_ = bounds[k0i + 1], bounds[k0i + 2]
            ed_rows[c][b] = (
                tr_s[s0:e0], ed_s[s0:e0], tr_s[s1_:e1_] - HML, ed_s[s1_:e1_])

    # ---- pass 2: shared per-block tile counts (max over cores)
    T0 = [0] * NBLK
    T1 = [0] * NBLK
    for b in range(NBLK):
        for c in range(NCORES):
            r0, _, r1, _ = ed_rows[c][b]
            T0[b] = max(T0[b], (len(r0) + 127) // 128)
            T1[b] = max(T1[b], (len(r1) + 127) // 128)
    T0 = [max(t, 1) for t in T0]
    T1 = [max(t, 1) for t in T1]
    assert max(max(T0), max(T1)) <= 16, (max(T0), max(T1))
    cfg["T0"], cfg["T1"] = T0, T1
    TMAXH0 = max(T0)
    TMAXH1 = max(T1)
    TMAX = max(a + b for a, b in zip(T0, T1))
    cfg["TMAXH0"], cfg["TMAXH1"], cfg["TMAX"] = TMAXH0, TMAXH1, TMAX
    W0, W1 = TMAXH0 * 8, TMAXH1 * 8

    # ---- pass 3: per-core arrays
    maps = []
    for c in range(NCORES):
        idxg = np.zeros((NBLK * 128, W0 + W1), np.int16)
        dst_col = np.full((NBLK * 128, TMAX), -1.0, np.float32)
        dst_row = np.full((NBLK, TMAX * 128), -1.0, np.float32)
        for b in range(NBLK):
            r0, d0, r1, d1 = ed_rows[c][b]
            t0, t1 = T0[b], T1[b]
            L0 = np.zeros(t0 * 128, np.int64)
            L0[: len(r0)] = r0
            L1 = np.zeros(t1 * 128, np.int64)
            L1[: len(r1)] = r1
            idxg[b * 128 : (b + 1) * 128, : t0 * 8] = _wrap16(L0)
            idxg[b * 128 : (b + 1) * 128, W0 : W0 + t1 * 8] = _wrap16(L1)
            dv = np.full((t0 + t1) * 128, -1.0, np.float32)
            dv[: len(d0)] = d0
            dv[t0 * 128 : t0 * 128 + len(d1)] = d1
            dst_row[b, : (t0 + t1) * 128] = dv
            dc = dv.reshape(t0 + t1, 128).T  # [128, T]
            dst_col[b * 128 : (b + 1) * 128, : t0 + t1] = dc

        # own x^T blocks (for the xr projection pass), in own-block order
        own_tabrows = np.asarray(tabrow_of_slot(cfg, c * NP + np.arange(NP)))
        xownT = (
            x_tab[own_tabrows]
            .reshape(NBLK, 128, Din)
            .transpose(0, 2, 1)
            .reshape(NBLK * Din, 128)
        ).astype(BF16)

        # pool mask [NP, G] over own slots
        pm = np.zeros((NP, G), np.float32)
        own_nodes = node_of_slot[c * NP : (c + 1) * NP]
        vv = own_nodes >= 0
        pm[np.arange(NP)[vv], batch[own_nodes[vv]]] = 1.0

        maps.append(
            dict(
                xfullT=xfullT,
                xownT=xownT,
                idxg=idxg,
                dst_col=dst_col.astype(BF16),
                dst_row=dst_row.astype(BF16),
                pool_mask=pm.astype(BF16),
            )
        )

    counts = np.bincount(batch, minlength=G).astype(np.float32)
    return maps, counts


def prep_weights(cfg, inp):
    w = {}
    for l in range(3):
        Wl = np.asarray(inp[f"Wl{l}"], np.float32)
        bl = np.asarray(inp[f"bl{l}"], np.float32)
        Wr = np.asarray(inp[f"Wr{l}"], np.float32)
        br = np.asarray(inp[f"br{l}"], np.float32)
        bo = np.asarray(inp[f"bias{l}"], np.float32)
        D = Wl.shape[1]
        # table holds x@Wl (no bias); xr bias = bl+br; out bias += bl
        w[f"wcat{l}"] = np.concatenate([Wl, Wr], axis=1).astype(BF16)  # [Din,2D]
        w[f"bias_r{l}"] = np.broadcast_to((bl + br)[None, :], (128, D)).copy()
        w[f"bias_out{l}"] = np.broadcast_to((bo + bl)[None, :], (128, D)).copy()
    w["iota_col"] = np.arange(128, dtype=np.float32)[:, None]
    TMAX = cfg["TMAX"]
    w["iota_rep"] = np.broadcast_to(
        np.arange(128, dtype=np.float32)[None, None, :], (128, TMAX, 128)
    ).reshape(128, TMAX * 128).astype(BF16)
    for l in range(3):
        D = [HIDDEN, HIDDEN, OUT_CH][l]
        w[f"att{l}r"] = np.broadcast_to(
            np.asarray(inp[f"att{l}"], np.float32).reshape(1, 1, D), (128, TMAX, D)
        ).reshape(128, TMAX * D).astype(BF16)
    w["ident"] = np.eye(128, dtype=np.float32)
    return w


# ---------------------------------------------------------------- device build
def build(cfg):
    from concourse import bass, bacc, mybir
    import concourse.tile as tile
    from concourse.tile import add_dep_helper

    F32 = mybir.dt.float32
    BF = mybir.dt.bfloat16
    I16 = mybir.dt.int16
    A = mybir.AluOpType
    ACTF = mybir.ActivationFunctionType

    NP, NBLK, TROWS, G = cfg["NP"], cfg["NBLK"], cfg["TROWS"], cfg["G"]
    CHA, CHB, GBLK, HML = cfg["CHA"], cfg["CHB"], cfg["GBLK"], cfg["HML"]
    T0, T1, TMAX = cfg["T0"], cfg["T1"], cfg["TMAX"]
    TMAXH0, TMAXH1 = cfg["TMAXH0"], cfg["TMAXH1"]
    W0, W1 = TMAXH0 * 8, TMAXH1 * 8
    Din0 = cfg["IN_CH"]
    LAYERS = cfg["LAYERS"]
    LAG = 2

    nc = bacc.Bacc(
        "TRN2",
        target_bir_lowering=False,
        debug=False,
        enable_asserts=False,
        num_devices=NCORES,
        num_swdge_queues=4,
    )

    ext = {}

    def ein(name, shape, dt):
        ext[name] = nc.dram_tensor(name, shape, dt, kind="ExternalInput").ap()
        return ext[name]

    xfullT = ein("xfullT", [(GBLK // 8) * Din0, 8 * 128], BF)
    xownT = ein("xownT", [NBLK * Din0, 128], BF)
    idxg_d = ein("idxg", [NBLK * 128, W0 + W1], I16)
    dst_col = ein("dst_col", [NBLK * 128, TMAX], BF)
    dst_row = ein("dst_row", [NBLK, TMAX * 128], BF)
    pool_mask = ein("pool_mask", [NP, G], BF)
    iota_col_d = ein("iota_col", [128, 1], F32)
    iota_rep_d = ein("iota_rep", [128, TMAX * 128], BF)
    ident_d = ein("ident", [128, 128], F32)
    wcat_d, biasr_d, att_d, biasout_d = [], [], [], []
    for l, (Din, D, H, C, _) in enumerate(LAYERS):
        wcat_d.append(ein(f"wcat{l}", [Din, 2 * D], BF))
        biasr_d.append(ein(f"bias_r{l}", [128, D], F32))
        att_d.append(ein(f"att{l}r", [128, TMAX * D], BF))
        biasout_d.append(ein(f"bias_out{l}", [128, D], F32))

    pool_out = nc.dram_tensor("pool_out", [G, OUT_CH], F32, kind="ExternalOutput").ap()

    # internal DRAM: tables are bf16, 128 cols even for layer 2 (gather rows
    # must be 256B multiples)
    tabs = []
    ccA, ccB = [None] * 3, [None] * 3
    for l in range(3):
        tabs.append(
            nc.dram_tensor(
                f"tab{l}", [TROWS, 128], BF, kind="Internal", addr_space="Shared"
            ).ap()
        )
        if l >= 1:
            ccA[l] = nc.dram_tensor(f"ccA{l}", [CHA * 128, 128], BF, kind="Internal").ap()
            ccB[l] = nc.dram_tensor(f"ccB{l}", [CHB * 128, 128], BF, kind="Internal").ap()

    from contextlib import ExitStack

    with tile.TileContext(nc) as tc, ExitStack() as pools:
        const = pools.enter_context(tc.tile_pool(name="const", bufs=1))
        s1 = pools.enter_context(tc.tile_pool(name="s1", bufs=LAG + 2))
        s1b = pools.enter_context(tc.tile_pool(name="s1b", bufs=2))
        s2 = pools.enter_context(tc.tile_pool(name="s2", bufs=2))
        nodep = pools.enter_context(tc.tile_pool(name="nodep", bufs=2))
        # PSUM: 8 banks x 2KB.  vexp 4 banks, agg 1, pam 1, pat 1, pool 1.
        psum_exp = pools.enter_context(tc.tile_pool(name="psum_exp", bufs=1, space="PSUM"))
        psum_agg = pools.enter_context(tc.tile_pool(name="psum_agg", bufs=1, space="PSUM"))
        psum_pam = pools.enter_context(tc.tile_pool(name="psum_pam", bufs=1, space="PSUM"))
        psum_pat = pools.enter_context(tc.tile_pool(name="psum_pat", bufs=1, space="PSUM"))
        psum_pool = pools.enter_context(tc.tile_pool(name="psum_pool", bufs=1, space="PSUM"))

        # persistent SBUF: xr tables (double-buffered across layers)
        xr_sb = [
            nc.alloc_sbuf_tensor(f"xr_sb{k}", [128, NBLK, HIDDEN], BF).ap()
            for k in range(2)
        ]

        def const_tile(shape, dt, src_ap, tag):
            t = const.tile(shape, dt, tag=tag)
            nc.sync.dma_start(out=t[:], in_=src_ap)
            return t

        ident = const_tile([128, 128], F32, ident_d[:], "ident")
        iota_c = const_tile([128, 1], F32, iota_col_d[:], "iotac")
        iota_rep = const_tile(
            [128, TMAX, 128], BF,
            iota_rep_d[:].rearrange("p (t q) -> p t q", t=TMAX), "iotarep")
        wcat_s, biasr_s, att_s, biasout_s = [], [], [], []
        for l, (Din, D, H, C, _) in enumerate(LAYERS):
            wcat_s.append(const_tile([Din, 2 * D], BF, wcat_d[l][:], f"wc{l}"))
            biasr_s.append(const_tile([128, D], F32, biasr_d[l][:], f"br{l}"))
            att_s.append(const_tile([128, TMAX * D], BF, att_d[l][:], f"at{l}"))
            biasout_s.append(const_tile([128, D], F32, biasout_d[l][:], f"bo{l}"))

        # ============ layer 0: local full-table build + own xr pass
        D0 = LAYERS[0][1]
        tab0_writes = []
        GRP = 8
        assert GBLK % GRP == 0
        for gg in range(GBLK // GRP):
            xT8 = nodep.tile([Din0, GRP, 128], BF, tag="t0_xT")
            nc.sync.dma_start(
                out=xT8[:],
                in_=xfullT[gg * Din0 : (gg + 1) * Din0, :],
            )
            vps8 = psum_exp.tile([128, GRP, D0], F32, tag="vexp")
            for k in range(GRP):
                nc.tensor.matmul(
                    out=vps8[:, k, :], lhsT=xT8[:, k, :], rhs=wcat_s[0][:, :D0],
                    start=True, stop=True
                )
            xl8 = nodep.tile([128, GRP, D0], BF, tag="t0_xl")
            nc.scalar.copy(out=xl8[:], in_=vps8[:])
            wi = nc.sync.dma_start(
                out=tabs[0][gg * GRP * 128 : (gg + 1) * GRP * 128, :].rearrange(
                    "(g p) d -> p g d", g=GRP
                ),
                in_=xl8[:],
            )
            tab0_writes.append(wi)

        for b in range(NBLK):
            xT = nodep.tile([Din0, 128], BF, tag="own_xT")
            nc.sync.dma_start(out=xT[:], in_=xownT[b * Din0 : (b + 1) * Din0, :])
            pr_f = psum_pam.tile([128, 2 * HIDDEN], F32, tag="pa_mm")
            pr = pr_f[:, :D0]
            nc.tensor.matmul(
                out=pr, lhsT=xT[:], rhs=wcat_s[0][:, D0:], start=True, stop=True
            )
            nc.vector.tensor_tensor(
                out=xr_sb[0][:, b, :D0], in0=pr, in1=biasr_s[0][:], op=A.add
            )

        # barrier proxy: one op depending on all tab0 writes; gathers dep on it
        barrier0 = nc.scalar.copy(out=ident[:1, :1], in_=ident[:1, :1])
        for wi in tab0_writes:
            add_dep_helper(barrier0.ins, wi.ins, sync=True, reason="tab0 done")

        # ============ layers
        ag_calls = {0: [barrier0]}  # per-layer list of deps for gathers

        for l, (Din, D, H, C, use_elu) in enumerate(LAYERS):
            HD = H + D
            xr_cur = xr_sb[l % 2]
            xr_nxt = xr_sb[(l + 1) % 2]
            gather_deps = ag_calls[l]
            if l < 2:
                pa_writesA, pa_writesB = [], []
            if l == 2:
                pool_ps = psum_pool.tile([G, OUT_CH], F32, tag="pool")

            state = {}

            def stage1(b, l=l, D=D, state=state,
                       xr_cur=xr_cur, gather_deps=gather_deps):
                t0, t1 = T0[b], T1[b]
                Tb = t0 + t1
                dcol = s1.tile([128, Tb, 1], BF, tag="dcol")
                nc.sync.dma_start(
                    out=dcol[:], in_=dst_col[b * 128 : (b + 1) * 128, :Tb]
                )
                idxt = s1.tile([128, W0 + W1], I16, tag="idxt")
                nc.sync.dma_start(
                    out=idxt[:], in_=idxg_d[b * 128 : (b + 1) * 128, :]
                )
                drep = s1b.tile([128, Tb * 128], BF, tag="drep")
                nc.sync.dma_start(
                    out=drep[:],
                    in_=dst_row[b : b + 1, : Tb * 128].to_broadcast([128, Tb * 128]),
                )
                # mT[q, (t,s)] = (dstrow(edge t*128+s) == q)
                mT = s1b.tile([128, Tb, 128], BF, tag="mT")
                nc.vector.tensor_scalar(
                    out=mT[:].rearrange("p t q -> p (t q)"), in0=drep[:],
                    scalar1=iota_c[:, :1], scalar2=None, op0=A.is_equal)
                # mE[p, t, j] = (dcol[p,t] == j)
                mE = s1.tile([128, Tb, 128], BF, tag="mE")
                nc.vector.tensor_tensor(
                    out=mE[:], in0=iota_rep[:, :Tb, :],
                    in1=dcol[:].to_broadcast([128, Tb, 128]), op=A.is_equal)
                # xr expansion per half into one PSUM tile + one copy
                v_all = s1.tile([128, Tb, D], BF, tag="v")
                for h, (tb, toff) in enumerate([(t0, 0), (t1, t0)]):
                    vps = psum_exp.tile([128, max(TMAXH0, TMAXH1, GRP), D0], F32,
                                        tag="vexp")
                    for t in range(tb):
                        nc.tensor.matmul(
                            out=vps[:, t, :D], lhsT=mT[:, toff + t, :],
                            rhs=xr_cur[:, b, :D], start=True, stop=True)
                    nc.scalar.copy(
                        out=v_all[:, toff : toff + tb, :], in_=vps[:, :tb, :D])
                # batched gathers: dma_gather per table half, chunked to <=8
                # tiles (1024 descs) -- the SWDGE ring holds 1024 descriptors
                g_all = s1.tile([128, Tb, 128], BF, tag="g")
                qn = b % 4
                for tb_h, toff, ioff, roff in (
                    (t0, 0, 0, 0), (t1, t0, W0, HML)):
                    done = 0
                    while done < tb_h:
                        ch = min(8, tb_h - done)
                        gi = nc.gpsimd.dma_gather(
                            out_ap=g_all[:, toff + done : toff + done + ch, :],
                            in_ap=tabs[l][roff : roff + HML, :],
                            idxs_ap=idxt[:, ioff + done * 8 : ioff + (done + ch) * 8],
                            num_idxs=ch * 128, num_idxs_reg=ch * 128,
                            elem_size=128, queue_num=qn)
                        for dep in gather_deps:
                            add_dep_helper(gi.ins, dep.ins, sync=True,
                                           reason="gather after table ready")
                        done += ch
                        qn = (qn + 1) % 4
                state[b] = (v_all, g_all, mE)

            def stage2(b, l=l, D=D, H=H, C=C, HD=HD, state=state,
                       xr_cur=xr_cur, xr_nxt=xr_nxt, use_elu=use_elu):
                t0, t1 = T0[b], T1[b]
                Tb = t0 + t1
                v_all, g_all, mE = state.pop(b)
                gD = g_all[:, :, :D]
                vs = s2.tile([128, Tb, D], BF, tag="vsum")
                nc.vector.tensor_tensor(
                    out=vs[:], in0=gD, in1=v_all[:], op=A.add)
                v4 = gD.rearrange("p t (h c) -> p t h c", h=H)
                vf = vs[:].rearrange("p t d -> p (t d)")
                l_all = s2.tile([128, Tb * D], BF, tag="lrelu")
                nc.vector.scalar_tensor_tensor(
                    out=l_all[:], in0=vf, scalar=NEG_SLOPE, in1=vf,
                    op0=A.mult, op1=A.max)
                p_all = s2.tile([128, Tb, H, C], BF, tag="patt")
                nc.vector.tensor_tensor(
                    out=p_all[:].rearrange("p t h c -> p (t h c)"), in0=l_all[:],
                    in1=att_s[l][:, : Tb * D], op=A.mult)
                lg = s2.tile([128, Tb, H], F32, tag="lg")
                nc.vector.tensor_reduce(
                    out=lg[:], in_=p_all[:], axis=mybir.AxisListType.X, op=A.add)
                e_t = s2.tile([128, Tb, H, 1], BF, tag="expv")
                nc.scalar.activation(out=e_t[:], in_=lg[:], func=ACTF.Exp)
                w_all = s2.tile([128, Tb, HD], BF, tag="wall")
                nc.scalar.copy(out=w_all[:, :, :H], in_=e_t[:])
                nc.vector.tensor_tensor(
                    out=w_all[:, :, H:].rearrange("p t (h c) -> p t h c", h=H),
                    in0=v4,
                    in1=e_t[:].to_broadcast([128, Tb, H, C]), op=A.mult)

                o_ps = psum_agg.tile([128, HD], F32, tag="agg")
                for t in range(Tb):
                    nc.tensor.matmul(
                        out=o_ps[:], lhsT=mE[:, t, :], rhs=w_all[:, t, :],
                        start=(t == 0), stop=(t == Tb - 1))

                dn = nodep.tile([128, H], F32, tag="dn")
                nc.vector.tensor_scalar(
                    out=dn[:], in0=o_ps[:, :H], scalar1=1e-30, scalar2=None, op0=A.add)
                rc = nodep.tile([128, H], F32, tag="rc")
                nc.vector.reciprocal(out=rc[:], in_=dn[:])
                onorm = nodep.tile([128, H, C], F32, tag="onorm")
                nc.vector.tensor_tensor(
                    out=onorm[:],
                    in0=o_ps[:, H:].rearrange("p (h c) -> p h c", h=H),
                    in1=rc[:].rearrange("p (h o) -> p h o", h=H).to_broadcast([128, H, C]),
                    op=A.mult)
                hb = nodep.tile([128, D], F32, tag="hb")
                nc.vector.tensor_tensor(
                    out=hb[:], in0=onorm[:].rearrange("p h c -> p (h c)"),
                    in1=biasout_s[l][:], op=A.add)
                if use_elu:
                    amax = nodep.tile([128, D], F32, tag="amax")
                    nc.vector.tensor_scalar(
                        out=amax[:], in0=hb[:], scalar1=0.0, scalar2=None, op0=A.max)
                    amin = nodep.tile([128, D], F32, tag="amin")
                    nc.vector.tensor_scalar(
                        out=amin[:], in0=hb[:], scalar1=0.0, scalar2=None, op0=A.min)
                    aexp = nodep.tile([128, D], F32, tag="aexp")
                    nc.scalar.activation(out=aexp[:], in_=amin[:], func=ACTF.Exp)
                    h_t = nodep.tile([128, D], F32, tag="h")
                    nc.vector.scalar_tensor_tensor(
                        out=h_t[:], in0=amax[:], scalar=-1.0, in1=aexp[:],
                        op0=A.add, op1=A.add)
                    # fused projection for layer l+1
                    Dn = LAYERS[l + 1][1]
                    tp = psum_pat.tile([128, 128], F32, tag="pa_tr")
                    nc.tensor.transpose(out=tp[:], in_=h_t[:], identity=ident[:])
                    hT = nodep.tile([128, 128], BF, tag="pa_hT")
                    nc.scalar.copy(out=hT[:], in_=tp[:])
                    pp_f = psum_pam.tile([128, 2 * HIDDEN], F32, tag="pa_mm")
                    pp = pp_f[:, : 2 * Dn]
                    nc.tensor.matmul(
                        out=pp, lhsT=hT[:], rhs=wcat_s[l + 1][:],
                        start=True, stop=True)
                    xl_t = nodep.tile([128, 128], BF, tag="pa_xl")
                    nc.scalar.copy(out=xl_t[:, :Dn], in_=pp[:, :Dn])
                    if Dn < 128:
                        nc.scalar.copy(out=xl_t[:, Dn:], in_=pp[:, :128 - Dn])
                    if b < CHA:
                        wi = nc.sync.dma_start(
                            out=ccA[l + 1][b * 128 : (b + 1) * 128, :], in_=xl_t[:])
                        pa_writesA.append(wi)
                    else:
                        wi = nc.sync.dma_start(
                            out=ccB[l + 1][(b - CHA) * 128 : (b - CHA + 1) * 128, :],
                            in_=xl_t[:])
                        pa_writesB.append(wi)
                    nc.vector.tensor_tensor(
                        out=xr_nxt[:, b, :Dn], in0=pp[:, Dn : 2 * Dn],
                        in1=biasr_s[l + 1][:], op=A.add)
                    if b == CHA - 1:
                        agA = nc.gpsimd.collective_compute(
                            "AllGather", A.bypass,
                            replica_groups=[list(range(NCORES))],
                            ins=[ccA[l + 1][:]],
                            outs=[tabs[l + 1][0 : NCORES * CHA * 128, :]],
                        )
                        for wi in pa_writesA:
                            add_dep_helper(agA.ins, wi.ins, sync=True, reason="agA")
                        ag_calls[l + 1] = [agA]
                    if b == NBLK - 1:
                        agB = nc.gpsimd.collective_compute(
                            "AllGather", A.bypass,
                            replica_groups=[list(range(NCORES))],
                            ins=[ccB[l + 1][:]],
                            outs=[tabs[l + 1][NCORES * CHA * 128 :, :]],
                        )
                        for wi in pa_writesB:
                            add_dep_helper(agB.ins, wi.ins, sync=True, reason="agB")
                        ag_calls[l + 1].append(agB)
                else:
                    h2b = nodep.tile([128, D], BF, tag="h2b")
                    nc.vector.tensor_scalar(
                        out=h2b[:], in0=hb[:], scalar1=0.0, scalar2=None, op0=A.add)
                    pm_t = nodep.tile([128, G], BF, tag="pmt")
                    nc.sync.dma_start(
                        out=pm_t[:], in_=pool_mask[b * 128 : (b + 1) * 128, :])
                    nc.tensor.matmul(
                        out=pool_ps[:], lhsT=pm_t[:], rhs=h2b[:],
                        start=(b == 0), stop=(b == NBLK - 1))

            for b in range(NBLK + LAG):
                if b < NBLK:
                    stage1(b)
                if b >= LAG:
                    stage2(b - LAG)

        pool_sb = nodep.tile([G, OUT_CH], F32, tag="poolsb")
        nc.scalar.copy(out=pool_sb[:], in_=pool_ps[:])
        nc.sync.dma_start(out=pool_out[:], in_=pool_sb[:])

    nc.compile()
    return nc


# ---------------------------------------------------------------- runner
_BUILD_CACHE = {}


def run(cfg, inp, trace=False):
    from concourse import bass_utils

    maps, counts = prep(cfg, np.asarray(inp["x"], np.float32), inp["edge_index"], inp["batch"])
    w = prep_weights(cfg, inp)
    for m in maps:
        m.update(w)

    key = (cfg["N"], cfg["G"], tuple(cfg["T0"]), tuple(cfg["T1"]))
    if key not in _BUILD_CACHE:
        _BUILD_CACHE[key] = build(cfg)
    nc = _BUILD_CACHE[key]

    res = bass_utils.run_bass_kernel_spmd(
        nc, maps, core_ids=list(range(NCORES)), trace=trace
    )
    total = np.zeros((cfg["G"], OUT_CH), np.float64)
    for k in range(NCORES):
        total += res.results[k]["pool_out"].astype(np.float64)
    out = (total / np.maximum(counts, 1.0)[:, None]).astype(np.float32)
    return out, res


def kernel(**inputs) -> np.ndarray:
    cfg = make_cfg()
    out, _ = run(cfg, inputs, trace=False)
    return out


# revision 26
# speedup vs baseline: 1.8446x; 1.1671x over previous
"""GATv2 3-layer GNN on 8 Trainium2 NeuronCores (Bass/Tile) — v5.

Key structure (per core):
  - Nodes are host-binned into 8*49=392 blocks of 128 slots with balanced
    in-degree per block.
  - The xl gather uses gpsimd dma_gather (one instruction per table-half
    per block, ~1us SWDGE each) instead of per-tile indirect DMAs
    (994ns fixed overhead each, 16 per block).  dma_gather indices are
    int16, so the 50176-row table is split in two halves; each block's
    edges are packed half-0-first into whole 128-slot tiles, padded with
    dummy row-0 gathers so every core runs identical shapes (SPMD).
  - Tables are bf16 (dma_gather needs 256B-multiple rows; also improves
    accuracy over fp8).  Layer 2 (D=64) pads table rows to 128 cols.
  - Layer 0: every core builds the FULL xl0 table locally from a
    pre-transposed bf16 copy of x (no AllGather for layer 0).
  - Layers 1,2: xl shards are exchanged with a 2-chunk AllGather
    (blocks 0..CHA-1 early, rest late) so most of the exchange hides
    behind phase-B work of the producing layer.
  - Phase B is software-pipelined: stage1 (mask build, xr-expand
    matmuls batched into one PSUM tile + one copy per half, dma_gather)
    runs LAG blocks ahead of stage2 (edge math, one-hot aggregation,
    node update + fused projection of the NEXT layer).
  - bias trick: the table holds x@Wl WITHOUT bias; bl is folded into
    xr's bias (v = xl'+xr' is unchanged) and into the output bias.
"""

import sys

if "/opt/trn_rl_repo" not in sys.path:
    sys.path.insert(0, "/opt/trn_rl_repo")

import numpy as np
import ml_dtypes

BF16 = ml_dtypes.bfloat16

NEG_SLOPE = 0.2
N_NODES = 50000
N_EDGES = 800000
N_GRAPHS = 64
IN_CH = 128
HIDDEN = 128
HEADS = 4
OUT_CH = 64
NCORES = 8


def make_cfg(n_nodes=N_NODES, n_graphs=N_GRAPHS, in_ch=IN_CH):
    npc = n_nodes // NCORES
    assert npc * NCORES == n_nodes
    nblk = (npc + 127) // 128
    np_pad = nblk * 128
    cha = max(1, (nblk * 3) // 5)  # early AG chunk (blocks [0, cha))
    trows = NCORES * np_pad
    return dict(
        N=n_nodes,
        G=n_graphs,
        NPC=npc,
        NP=np_pad,
        NBLK=nblk,
        CHA=cha,
        CHB=nblk - cha,
        GBLK=NCORES * nblk,
        TROWS=trows,
        HML=trows // 2,
        IN_CH=in_ch,
        T0=None,  # per-block tile counts for table half 0 (list, len NBLK)
        T1=None,
        LAYERS=[
            (in_ch, HIDDEN, HEADS, HIDDEN // HEADS, True),
            (HIDDEN, HIDDEN, HEADS, HIDDEN // HEADS, True),
            (HIDDEN, OUT_CH, 1, OUT_CH, False),
        ],
    )


# ---------------------------------------------------------------- host prep
def _balanced_bins(deg, nbins, binsz):
    """Assign nodes to bins (each bin holds exactly binsz nodes) minimizing
    max total degree per bin.  Greedy: degree-desc, min-load non-full bin.
    Returns slot_of[node] = bin*binsz + position."""
    import heapq

    n = deg.shape[0]
    order = np.argsort(-deg, kind="stable")
    heap = [(0, b) for b in range(nbins)]
    heapq.heapify(heap)
    fill = np.zeros(nbins, np.int64)
    load = np.zeros(nbins, np.int64)
    slot_of = np.empty(n, np.int64)
    for nd in order:
        while True:
            l, b = heapq.heappop(heap)
            if fill[b] < binsz:
                break
        slot_of[nd] = b * binsz + fill[b]
        fill[b] += 1
        load[b] += deg[nd]
        if fill[b] < binsz:
            heapq.heappush(heap, (load[b], b))
    return slot_of, int(load.max())


def tabrow_of_slot(cfg, slot):
    """Map global slot id -> table row (2-chunk AllGather layout)."""
    NP, NBLK, CHA = cfg["NP"], cfg["NBLK"], cfg["CHA"]
    c = slot // NP
    loc = slot % NP
    b = loc // 128
    r = loc % 128
    rowsA = NCORES * CHA * 128
    return np.where(
        b < CHA,
        c * CHA * 128 + b * 128 + r,
        rowsA + c * (NBLK - CHA) * 128 + (b - CHA) * 128 + r,
    )


def _wrap16(lst):
    """dma_gather index layout: idx k -> [k%16, k//16], replicated x8."""
    n = lst.shape[0]
    assert n % 16 == 0
    w = lst.reshape(n // 16, 16).T  # [16, W]
    return np.tile(w, (8, 1)).astype(np.int16)  # [128, W]


def prep(cfg, x, edge_index, batch):
    NPC, NP, NBLK, G, CHA = cfg["NPC"], cfg["NP"], cfg["NBLK"], cfg["G"], cfg["CHA"]
    GBLK, HML = cfg["GBLK"], cfg["HML"]
    Din = cfg["IN_CH"]
    src = np.asarray(edge_index[0], dtype=np.int64)
    dst = np.asarray(edge_index[1], dtype=np.int64)
    batch = np.asarray(batch, dtype=np.int64)
    x = np.asarray(x, dtype=np.float32)
    N = x.shape[0]

    deg = np.bincount(dst, minlength=N)
    slot_of, maxload = _balanced_bins(deg, GBLK, 128)

    node_of_slot = np.full(GBLK * 128, -1, np.int64)
    node_of_slot[slot_of] = np.arange(N)

    # permuted x, laid out in TABLE-ROW block order, transposed per block
    x_slot = np.zeros((GBLK * 128, Din), np.float32)
    valid = node_of_slot >= 0
    x_slot[valid] = x[node_of_slot[valid]]
    tabrow = np.asarray(tabrow_of_slot(cfg, np.arange(GBLK * 128)))
    x_tab = np.zeros_like(x_slot)
    x_tab[tabrow] = x_slot
    assert GBLK % 8 == 0
    xfullT = (
        x_tab.reshape(GBLK // 8, 8, 128, Din)
        .transpose(0, 3, 1, 2)
        .reshape((GBLK // 8) * Din, 8 * 128)
    ).astype(BF16)

    # edges
    sd = slot_of[dst]
    ss = slot_of[src]
    trow = np.asarray(tabrow_of_slot(cfg, ss))
    core_of = sd // NP
    dloc = sd % NP
    bloc = dloc // 128
    drow = dloc % 128

    # ---- pass 1: per (core, block) edge lists split by table half
    half = (trow >= HML).astype(np.int64)
    key = (core_of * NBLK + bloc) * 2 + half
    order = np.argsort(key, kind="stable")
    ks = key[order]
    tr_s = trow[order]
    ed_s = drow[order]
    bounds = np.searchsorted(ks, np.arange(NCORES * NBLK * 2 + 1))
    ed_rows = [[None] * NBLK for _ in range(NCORES)]  # (rows0, d0, rows1, d1)
    for c in range(NCORES):
        for b in range(NBLK):
            k0i = (c * NBLK + b) * 2
            s0, e0 = bounds[k0i], bounds[k0i + 1]
            s1_, e1_ = bounds[k0i + 1], bounds[k0i + 2]
            ed_rows[c][b] = (
                tr_s[s0:e0], ed_s[s0:e0], tr_s[s1_:e1_] - HML, ed_s[s1_:e1_])

    # ---- pass 2: shared per-block tile counts (max over cores)
    T0 = [0] * NBLK
    T1 = [0] * NBLK
    for b in range(NBLK):
        for c in range(NCORES):
            r0, _, r1, _ = ed_rows[c][b]
            T0[b] = max(T0[b], (len(r0) + 127) // 128)
            T1[b] = max(T1[b], (len(r1) + 127) // 128)
    T0 = [max(t, 1) for t in T0]
    T1 = [max(t, 1) for t in T1]
    assert max(max(T0), max(T1)) <= 16, (max(T0), max(T1))
    cfg["T0"], cfg["T1"] = T0, T1
    TMAXH0 = max(T0)
    TMAXH1 = max(T1)
    TMAX = max(a + b for a, b in zip(T0, T1))
    cfg["TMAXH0"], cfg["TMAXH1"], cfg["TMAX"] = TMAXH0, TMAXH1, TMAX
    W0, W1 = TMAXH0 * 8, TMAXH1 * 8

    # ---- pass 3: per-core arrays
    maps = []
    F8 = ml_dtypes.float8_e4m3
    ar128 = np.arange(128, dtype=np.int64)
    for c in range(NCORES):
        idxg = np.zeros((NBLK * 128, W0 + W1), np.int16)
        mT_h = np.zeros((NBLK * 128, TMAX * 128), F8)
        mE_h = np.zeros((NBLK * 128, TMAX * 128), F8)
        for b in range(NBLK):
            r0, d0, r1, d1 = ed_rows[c][b]
            t0, t1 = T0[b], T1[b]
            L0 = np.zeros(t0 * 128, np.int64)
            L0[: len(r0)] = r0
            L1 = np.zeros(t1 * 128, np.int64)
            L1[: len(r1)] = r1
            idxg[b * 128 : (b + 1) * 128, : t0 * 8] = _wrap16(L0)
            idxg[b * 128 : (b + 1) * 128, W0 : W0 + t1 * 8] = _wrap16(L1)
            Tb = t0 + t1
            dv = np.full(Tb * 128, -1, np.int64)
            dv[: len(d0)] = d0
            dv[t0 * 128 : t0 * 128 + len(d1)] = d1
            mT_h[b * 128 : (b + 1) * 128, : Tb * 128] = (
                dv[None, :] == ar128[:, None]).astype(F8)
            dc = dv.reshape(Tb, 128).T  # [128(p), Tb]
            mE_h[b * 128 : (b + 1) * 128, : Tb * 128] = (
                dc[:, :, None] == ar128[None, None, :]).reshape(128, Tb * 128).astype(F8)

        # own x^T blocks (for the xr projection pass), in own-block order
        own_tabrows = np.asarray(tabrow_of_slot(cfg, c * NP + np.arange(NP)))
        xownT = (
            x_tab[own_tabrows]
            .reshape(NBLK, 128, Din)
            .transpose(0, 2, 1)
            .reshape(NBLK * Din, 128)
        ).astype(BF16)

        # pool mask [NP, G] over own slots
        pm = np.zeros((NP, G), np.float32)
        own_nodes = node_of_slot[c * NP : (c + 1) * NP]
        vv = own_nodes >= 0
        pm[np.arange(NP)[vv], batch[own_nodes[vv]]] = 1.0

        maps.append(
            dict(
                xfullT=xfullT,
                xownT=xownT,
                idxg=idxg,
                mT_h=mT_h,
                mE_h=mE_h,
                pool_mask=pm.astype(BF16),
            )
        )

    counts = np.bincount(batch, minlength=G).astype(np.float32)
    return maps, counts


def prep_weights(cfg, inp):
    w = {}
    for l in range(3):
        Wl = np.asarray(inp[f"Wl{l}"], np.float32)
        bl = np.asarray(inp[f"bl{l}"], np.float32)
        Wr = np.asarray(inp[f"Wr{l}"], np.float32)
        br = np.asarray(inp[f"br{l}"], np.float32)
        bo = np.asarray(inp[f"bias{l}"], np.float32)
        D = Wl.shape[1]
        # table holds x@Wl (no bias); xr bias = bl+br; out bias += bl
        w[f"wcat{l}"] = np.concatenate([Wl, Wr], axis=1).astype(BF16)  # [Din,2D]
        w[f"bias_r{l}"] = np.broadcast_to((bl + br)[None, :], (128, D)).copy()
        w[f"bias_out{l}"] = np.broadcast_to((bo + bl)[None, :], (128, D)).copy()
    TMAX = cfg["TMAX"]
    for l in range(3):
        D = [HIDDEN, HIDDEN, OUT_CH][l]
        w[f"att{l}r"] = np.broadcast_to(
            np.asarray(inp[f"att{l}"], np.float32).reshape(1, 1, D), (128, TMAX, D)
        ).reshape(128, TMAX * D).astype(BF16)
    w["ident"] = np.eye(128, dtype=np.float32)
    return w


# ---------------------------------------------------------------- device build
def build(cfg):
    from concourse import bass, bacc, mybir
    import concourse.tile as tile
    from concourse.tile import add_dep_helper

    F32 = mybir.dt.float32
    BF = mybir.dt.bfloat16
    F8 = mybir.dt.float8e4
    I16 = mybir.dt.int16
    A = mybir.AluOpType
    ACTF = mybir.ActivationFunctionType

    NP, NBLK, TROWS, G = cfg["NP"], cfg["NBLK"], cfg["TROWS"], cfg["G"]
    CHA, CHB, GBLK, HML = cfg["CHA"], cfg["CHB"], cfg["GBLK"], cfg["HML"]
    T0, T1, TMAX = cfg["T0"], cfg["T1"], cfg["TMAX"]
    TMAXH0, TMAXH1 = cfg["TMAXH0"], cfg["TMAXH1"]
    W0, W1 = TMAXH0 * 8, TMAXH1 * 8
    Din0 = cfg["IN_CH"]
    LAYERS = cfg["LAYERS"]
    LAG = 2

    nc = bacc.Bacc(
        "TRN2",
        target_bir_lowering=False,
        debug=False,
        enable_asserts=False,
        num_devices=NCORES,
        num_swdge_queues=4,
    )

    ext = {}

    def ein(name, shape, dt):
        ext[name] = nc.dram_tensor(name, shape, dt, kind="ExternalInput").ap()
        return ext[name]

    xfullT = ein("xfullT", [(GBLK // 8) * Din0, 8 * 128], BF)
    xownT = ein("xownT", [NBLK * Din0, 128], BF)
    idxg_d = ein("idxg", [NBLK * 128, W0 + W1], I16)
    mT_d = ein("mT_h", [NBLK * 128, TMAX * 128], F8)
    mE_d = ein("mE_h", [NBLK * 128, TMAX * 128], F8)
    pool_mask = ein("pool_mask", [NP, G], BF)
    ident_d = ein("ident", [128, 128], F32)
    wcat_d, biasr_d, att_d, biasout_d = [], [], [], []
    for l, (Din, D, H, C, _) in enumerate(LAYERS):
        wcat_d.append(ein(f"wcat{l}", [Din, 2 * D], BF))
        biasr_d.append(ein(f"bias_r{l}", [128, D], F32))
        att_d.append(ein(f"att{l}r", [128, TMAX * D], BF))
        biasout_d.append(ein(f"bias_out{l}", [128, D], F32))

    pool_out = nc.dram_tensor("pool_out", [G, OUT_CH], F32, kind="ExternalOutput").ap()

    # internal DRAM: tables are bf16, 128 cols even for layer 2 (gather rows
    # must be 256B multiples)
    tabs = []
    ccA, ccB = [None] * 3, [None] * 3
    for l in range(3):
        tabs.append(
            nc.dram_tensor(
                f"tab{l}", [TROWS, 128], BF, kind="Internal", addr_space="Shared"
            ).ap()
        )
        if l >= 1:
            ccA[l] = nc.dram_tensor(f"ccA{l}", [CHA * 128, 128], BF, kind="Internal").ap()
            ccB[l] = nc.dram_tensor(f"ccB{l}", [CHB * 128, 128], BF, kind="Internal").ap()

    from contextlib import ExitStack

    with tile.TileContext(nc) as tc, ExitStack() as pools:
        const = pools.enter_context(tc.tile_pool(name="const", bufs=1))
        s1 = pools.enter_context(tc.tile_pool(name="s1", bufs=LAG + 2))
        s1b = pools.enter_context(tc.tile_pool(name="s1b", bufs=2))
        s2 = pools.enter_context(tc.tile_pool(name="s2", bufs=2))
        nodep = pools.enter_context(tc.tile_pool(name="nodep", bufs=2))
        # PSUM: 8 banks x 2KB.  vexp 4 banks, agg 1, pam 1, pat 1, pool 1.
        psum_exp = pools.enter_context(tc.tile_pool(name="psum_exp", bufs=1, space="PSUM"))
        psum_agg = pools.enter_context(tc.tile_pool(name="psum_agg", bufs=1, space="PSUM"))
        psum_pam = pools.enter_context(tc.tile_pool(name="psum_pam", bufs=1, space="PSUM"))
        psum_pat = pools.enter_context(tc.tile_pool(name="psum_pat", bufs=1, space="PSUM"))
        psum_pool = pools.enter_context(tc.tile_pool(name="psum_pool", bufs=1, space="PSUM"))

        # persistent SBUF: xr tables (double-buffered across layers)
        xr_sb = [
            nc.alloc_sbuf_tensor(f"xr_sb{k}", [128, NBLK, HIDDEN], BF).ap()
            for k in range(2)
        ]

        def const_tile(shape, dt, src_ap, tag):
            t = const.tile(shape, dt, tag=tag)
            nc.sync.dma_start(out=t[:], in_=src_ap)
            return t

        ident = const_tile([128, 128], F32, ident_d[:], "ident")
        wcat_s, biasr_s, att_s, biasout_s = [], [], [], []
        for l, (Din, D, H, C, _) in enumerate(LAYERS):
            wcat_s.append(const_tile([Din, 2 * D], BF, wcat_d[l][:], f"wc{l}"))
            biasr_s.append(const_tile([128, D], F32, biasr_d[l][:], f"br{l}"))
            att_s.append(const_tile([128, TMAX * D], BF, att_d[l][:], f"at{l}"))
            biasout_s.append(const_tile([128, D], F32, biasout_d[l][:], f"bo{l}"))

        # ============ layer 0: local full-table build + own xr pass
        D0 = LAYERS[0][1]
        tab0_writes = []
        GRP = 8
        assert GBLK % GRP == 0
        for gg in range(GBLK // GRP):
            xT8 = nodep.tile([Din0, GRP, 128], BF, tag="t0_xT")
            nc.sync.dma_start(
                out=xT8[:],
                in_=xfullT[gg * Din0 : (gg + 1) * Din0, :],
            )
            vps8 = psum_exp.tile([128, GRP, D0], F32, tag="vexp")
            for k in range(GRP):
                nc.tensor.matmul(
                    out=vps8[:, k, :], lhsT=xT8[:, k, :], rhs=wcat_s[0][:, :D0],
                    start=True, stop=True
                )
            xl8 = nodep.tile([128, GRP, D0], BF, tag="t0_xl")
            nc.scalar.copy(out=xl8[:], in_=vps8[:])
            wi = nc.sync.dma_start(
                out=tabs[0][gg * GRP * 128 : (gg + 1) * GRP * 128, :].rearrange(
                    "(g p) d -> p g d", g=GRP
                ),
                in_=xl8[:],
            )
            tab0_writes.append(wi)

        for b in range(NBLK):
            xT = nodep.tile([Din0, 128], BF, tag="own_xT")
            nc.sync.dma_start(out=xT[:], in_=xownT[b * Din0 : (b + 1) * Din0, :])
            pr_f = psum_pam.tile([128, 2 * HIDDEN], F32, tag="pa_mm")
            pr = pr_f[:, :D0]
            nc.tensor.matmul(
                out=pr, lhsT=xT[:], rhs=wcat_s[0][:, D0:], start=True, stop=True
            )
            nc.vector.tensor_tensor(
                out=xr_sb[0][:, b, :D0], in0=pr, in1=biasr_s[0][:], op=A.add
            )

        # barrier proxy: one op depending on all tab0 writes; gathers dep on it
        barrier0 = nc.scalar.copy(out=ident[:1, :1], in_=ident[:1, :1])
        for wi in tab0_writes:
            add_dep_helper(barrier0.ins, wi.ins, sync=True, reason="tab0 done")

        # ============ layers
        ag_calls = {0: [barrier0]}  # per-layer list of deps for gathers

        for l, (Din, D, H, C, use_elu) in enumerate(LAYERS):
            HD = H + D
            xr_cur = xr_sb[l % 2]
            xr_nxt = xr_sb[(l + 1) % 2]
            gather_deps = ag_calls[l]
            if l < 2:
                pa_writesA, pa_writesB = [], []
            if l == 2:
                pool_ps = psum_pool.tile([G, OUT_CH], F32, tag="pool")

            state = {}

            def stage1(b, l=l, D=D, state=state,
                       xr_cur=xr_cur, gather_deps=gather_deps):
                t0, t1 = T0[b], T1[b]
                Tb = t0 + t1
                idxt = s1.tile([128, W0 + W1], I16, tag="idxt")
                nc.sync.dma_start(
                    out=idxt[:], in_=idxg_d[b * 128 : (b + 1) * 128, :]
                )
                # host-precomputed one-hot masks (fp8): mT for xr-expansion,
                # mE for the dst aggregation
                mT = s1b.tile([128, Tb, 128], F8, tag="mT")
                nc.sync.dma_start(
                    out=mT[:].rearrange("p t q -> p (t q)"),
                    in_=mT_d[b * 128 : (b + 1) * 128, : Tb * 128])
                mE = s1.tile([128, Tb, 128], F8, tag="mE")
                nc.sync.dma_start(
                    out=mE[:].rearrange("p t q -> p (t q)"),
                    in_=mE_d[b * 128 : (b + 1) * 128, : Tb * 128])
                # xr expansion per half into one PSUM tile + one copy
                v_all = s1.tile([128, Tb, D], BF, tag="v")
                for h, (tb, toff) in enumerate([(t0, 0), (t1, t0)]):
                    vps = psum_exp.tile([128, max(TMAXH0, TMAXH1, GRP), D0], F32,
                                        tag="vexp")
                    for t in range(tb):
                        nc.tensor.matmul(
                            out=vps[:, t, :D], lhsT=mT[:, toff + t, :],
                            rhs=xr_cur[:, b, :D], start=True, stop=True)
                    nc.scalar.copy(
                        out=v_all[:, toff : toff + tb, :], in_=vps[:, :tb, :D])
                # batched gathers: dma_gather per table half, chunked to <=8
                # tiles (1024 descs) -- the SWDGE ring holds 1024 descriptors
                g_all = s1.tile([128, Tb, 128], BF, tag="g")
                qn = b % 4
                for tb_h, toff, ioff, roff in (
                    (t0, 0, 0, 0), (t1, t0, W0, HML)):
                    done = 0
                    while done < tb_h:
                        ch = min(8, tb_h - done)
                        gi = nc.gpsimd.dma_gather(
                            out_ap=g_all[:, toff + done : toff + done + ch, :],
                            in_ap=tabs[l][roff : roff + HML, :],
                            idxs_ap=idxt[:, ioff + done * 8 : ioff + (done + ch) * 8],
                            num_idxs=ch * 128, num_idxs_reg=ch * 128,
                            elem_size=128, queue_num=qn)
                        for dep in gather_deps:
                            add_dep_helper(gi.ins, dep.ins, sync=True,
                                           reason="gather after table ready")
                        done += ch
                        qn = (qn + 1) % 4
                state[b] = (v_all, g_all, mE)

            def stage2(b, l=l, D=D, H=H, C=C, HD=HD, state=state,
                       xr_cur=xr_cur, xr_nxt=xr_nxt, use_elu=use_elu):
                t0, t1 = T0[b], T1[b]
                Tb = t0 + t1
                v_all, g_all, mE = state.pop(b)
                gD = g_all[:, :, :D]
                vs = s2.tile([128, Tb, D], BF, tag="vsum")
                nc.vector.tensor_tensor(
                    out=vs[:], in0=gD, in1=v_all[:], op=A.add)
                v4 = gD.rearrange("p t (h c) -> p t h c", h=H)
                vf = vs[:].rearrange("p t d -> p (t d)")
                l_all = s2.tile([128, Tb * D], BF, tag="lrelu")
                nc.vector.scalar_tensor_tensor(
                    out=l_all[:], in0=vf, scalar=NEG_SLOPE, in1=vf,
                    op0=A.mult, op1=A.max)
                p_all = s2.tile([128, Tb, H, C], BF, tag="patt")
                nc.vector.tensor_tensor(
                    out=p_all[:].rearrange("p t h c -> p (t h c)"), in0=l_all[:],
                    in1=att_s[l][:, : Tb * D], op=A.mult)
                lg = s2.tile([128, Tb, H], F32, tag="lg")
                nc.vector.tensor_reduce(
                    out=lg[:], in_=p_all[:], axis=mybir.AxisListType.X, op=A.add)
                e_t = s2.tile([128, Tb, H, 1], BF, tag="expv")
                nc.scalar.activation(out=e_t[:], in_=lg[:], func=ACTF.Exp)
                w_all = s2.tile([128, Tb, HD], BF, tag="wall")
                nc.scalar.copy(out=w_all[:, :, :H], in_=e_t[:])
                nc.vector.tensor_tensor(
                    out=w_all[:, :, H:].rearrange("p t (h c) -> p t h c", h=H),
                    in0=v4,
                    in1=e_t[:].to_broadcast([128, Tb, H, C]), op=A.mult)

                o_ps = psum_agg.tile([128, HD], F32, tag="agg")
                for t in range(Tb):
                    nc.tensor.matmul(
                        out=o_ps[:], lhsT=mE[:, t, :], rhs=w_all[:, t, :],
                        start=(t == 0), stop=(t == Tb - 1))

                dn = nodep.tile([128, H], F32, tag="dn")
                nc.vector.tensor_scalar(
                    out=dn[:], in0=o_ps[:, :H], scalar1=1e-30, scalar2=None, op0=A.add)
                rc = nodep.tile([128, H], F32, tag="rc")
                nc.vector.reciprocal(out=rc[:], in_=dn[:])
                onorm = nodep.tile([128, H, C], F32, tag="onorm")
                nc.vector.tensor_tensor(
                    out=onorm[:],
                    in0=o_ps[:, H:].rearrange("p (h c) -> p h c", h=H),
                    in1=rc[:].rearrange("p (h o) -> p h o", h=H).to_broadcast([128, H, C]),
                    op=A.mult)
                hb = nodep.tile([128, D], F32, tag="hb")
                nc.vector.tensor_tensor(
                    out=hb[:], in0=onorm[:].rearrange("p h c -> p (h c)"),
                    in1=biasout_s[l][:], op=A.add)
                if use_elu:
                    amax = nodep.tile([128, D], F32, tag="amax")
                    nc.vector.tensor_scalar(
                        out=amax[:], in0=hb[:], scalar1=0.0, scalar2=None, op0=A.max)
                    amin = nodep.tile([128, D], F32, tag="amin")
                    nc.vector.tensor_scalar(
                        out=amin[:], in0=hb[:], scalar1=0.0, scalar2=None, op0=A.min)
                    aexp = nodep.tile([128, D], F32, tag="aexp")
                    nc.scalar.activation(out=aexp[:], in_=amin[:], func=ACTF.Exp)
                    h_t = nodep.tile([128, D], F32, tag="h")
                    nc.vector.scalar_tensor_tensor(
                        out=h_t[:], in0=amax[:], scalar=-1.0, in1=aexp[:],
                        op0=A.add, op1=A.add)
                    # fused projection for layer l+1
                    Dn = LAYERS[l + 1][1]
                    tp = psum_pat.tile([128, 128], F32, tag="pa_tr")
                    nc.tensor.transpose(out=tp[:], in_=h_t[:], identity=ident[:])
                    hT = nodep.tile([128, 128], BF, tag="pa_hT")
                    nc.scalar.copy(out=hT[:], in_=tp[:])
                    pp_f = psum_pam.tile([128, 2 * HIDDEN], F32, tag="pa_mm")
                    pp = pp_f[:, : 2 * Dn]
                    nc.tensor.matmul(
                        out=pp, lhsT=hT[:], rhs=wcat_s[l + 1][:],
                        start=True, stop=True)
                    xl_t = nodep.tile([128, 128], BF, tag="pa_xl")
                    nc.scalar.copy(out=xl_t[:, :Dn], in_=pp[:, :Dn])
                    if Dn < 128:
                        nc.scalar.copy(out=xl_t[:, Dn:], in_=pp[:, :128 - Dn])
                    if b < CHA:
                        wi = nc.sync.dma_start(
                            out=ccA[l + 1][b * 128 : (b + 1) * 128, :], in_=xl_t[:])
                        pa_writesA.append(wi)
                    else:
                        wi = nc.sync.dma_start(
                            out=ccB[l + 1][(b - CHA) * 128 : (b - CHA + 1) * 128, :],
                            in_=xl_t[:])
                        pa_writesB.append(wi)
                    nc.vector.tensor_tensor(
                        out=xr_nxt[:, b, :Dn], in0=pp[:, Dn : 2 * Dn],
                        in1=biasr_s[l + 1][:], op=A.add)
                    if b == CHA - 1:
                        agA = nc.gpsimd.collective_compute(
                            "AllGather", A.bypass,
                            replica_groups=[list(range(NCORES))],
                            ins=[ccA[l + 1][:]],
                            outs=[tabs[l + 1][0 : NCORES * CHA * 128, :]],
                        )
                        for wi in pa_writesA:
                            add_dep_helper(agA.ins, wi.ins, sync=True, reason="agA")
                        ag_calls[l + 1] = [agA]
                    if b == NBLK - 1:
                        agB = nc.gpsimd.collective_compute(
                            "AllGather", A.bypass,
                            replica_groups=[list(range(NCORES))],
                            ins=[ccB[l + 1][:]],
                            outs=[tabs[l + 1][NCORES * CHA * 128 :, :]],
                        )
                        for wi in pa_writesB:
                            add_dep_helper(agB.ins, wi.ins, sync=True, reason="agB")
                        ag_calls[l + 1].append(agB)
                else:
                    h2b = nodep.tile([128, D], BF, tag="h2b")
                    nc.vector.tensor_scalar(
                        out=h2b[:], in0=hb[:], scalar1=0.0, scalar2=None, op0=A.add)
                    pm_t = nodep.tile([128, G], BF, tag="pmt")
                    nc.sync.dma_start(
                        out=pm_t[:], in_=pool_mask[b * 128 : (b + 1) * 128, :])
                    nc.tensor.matmul(
                        out=pool_ps[:], lhsT=pm_t[:], rhs=h2b[:],
                        start=(b == 0), stop=(b == NBLK - 1))

            for b in range(NBLK + LAG):
                if b < NBLK:
                    stage1(b)
                if b >= LAG:
                    stage2(b - LAG)

        pool_sb = nodep.tile([G, OUT_CH], F32, tag="poolsb")
        nc.scalar.copy(out=pool_sb[:], in_=pool_ps[:])
        nc.sync.dma_start(out=pool_out[:], in_=pool_sb[:])

    nc.compile()
    return nc


# ---------------------------------------------------------------- runner
_BUILD_CACHE = {}


def run(cfg, inp, trace=False):
    from concourse import bass_utils

    maps, counts = prep(cfg, np.asarray(inp["x"], np.float32), inp["edge_index"], inp["batch"])
    w = prep_weights(cfg, inp)
    for m in maps:
        m.update(w)

    key = (cfg["N"], cfg["G"], tuple(cfg["T0"]), tuple(cfg["T1"]))
    if key not in _BUILD_CACHE:
        _BUILD_CACHE[key] = build(cfg)
    nc = _BUILD_CACHE[key]

    res = bass_utils.run_bass_kernel_spmd(
        nc, maps, core_ids=list(range(NCORES)), trace=trace
    )
    total = np.zeros((cfg["G"], OUT_CH), np.float64)
    for k in range(NCORES):
        total += res.results[k]["pool_out"].astype(np.float64)
    out = (total / np.maximum(counts, 1.0)[:, None]).astype(np.float32)
    return out, res


def kernel(**inputs) -> np.ndarray:
    cfg = make_cfg()
    out, _ = run(cfg, inputs, trace=False)
    return out
